# revision 28
# baseline (speedup 1.0000x reference)
"""Trainium2 Bass kernel for nn_MHC (dense transformer block: QKV -> causal
attention -> conv1d(k=3) -> causal attention (same K/V) -> out proj).

Sharding over 8 NeuronCores: data-parallel on batch (2) x tensor-parallel on
heads (16 heads -> 4 per core). Cores 0-3 own batch 0, cores 4-7 batch 1.
Per-token-block AllGather (groups of 4) exchanges attention-1 context (fp8
payload) so each core can run the channel-mixing conv for its own output
channels; gathers issue immediately after each block's normalize and overlap
the next block's attention compute.

fp8 (e4m3) DoubleRow matmuls carry the q/k projections, the conv1d, and the
attention-1 context accumulation (2-4x bf16 PE throughput); scores, v, ctx2
and the out projection stay bf16 for accuracy. Softmax denominators ride as
64 broadcast "ones" rows in each ctx matmul's stationary operand, so
normalization is a PSUM-direct copy/reciprocal/multiply on the vector engine
and the scalar engine runs exp only.
"""

import math

import numpy as np
import ml_dtypes

import concourse.bacc as bacc
import concourse.mybir as mybir
import concourse.tile as tile
from concourse.bass import ts
from concourse.bass_utils import run_bass_kernel_spmd

# Problem shapes (hardcoded per contract)
B, S, D = 2, 2048, 1024
H, DH = 16, 64
N_CORES = 8
HPC = 4          # heads per core
CL = HPC * DH    # 256 local channels
KT = D // 128    # 8 k-tiles over the model dim
KTP = KT // 2    # 4 fp8 double-row k-tile pairs
NJ = S // 512    # 4 t-blocks of 512
NS = S // 128    # 16 s-tiles of 128
GROUPS = [[0, 1, 2, 3], [4, 5, 6, 7]]

F32 = mybir.dt.float32
BF16 = mybir.dt.bfloat16
F8 = mybir.dt.float8e4
EXP = mybir.ActivationFunctionType.Exp
MULT = mybir.AluOpType.mult
ADD = mybir.AluOpType.add
DR = mybir.MatmulPerfMode.DoubleRow

E4NP = ml_dtypes.float8_e4m3
BFNP = ml_dtypes.bfloat16

P8S = 2.0        # fp8 scale on exp(score) in attention 1 (headroom for the
                 # below-diagonal scores that tri masks AFTER exp: fp8
                 # overflow there would turn the masked zeros into NaN)
V8S = 32.0       # fp8 scale on v for the attention-1 ctx matmul
ONE8 = 0.5       # ones-column value in v8
CTXGS = 64.0     # scale of the gathered fp8 ctx: P8S*V8S / (P8S*ONE8)
# ctx1 psum = (8p)(32v) = 256*sum(pv); den rows = (8p)(0.5) = 4*sum(p);
# evacuate-multiply by 1/denrows -> 64 * ctx = CTXGS * ctx.

# dsc columns (per-core dynamic constants, broadcast to 128 partitions)
DSC_Q = 0        # 2^-(ex+eq)*2^eq8: q-proj psum -> fp8 q
DSC_K = 1        # 2^-(ex+ek)*2^ek8: k-proj psum -> fp8 k
DSC_CV = 2       # 2^-(ecw+6)*2^eq28: conv psum -> fp8 q2
DSC_LNP8 = 3     # ln(P8S): exp bias for attention 1
DSC_ZERO = 4     # 0.0: exp bias for attention 2
DSC_V8 = 5       # V8S
DSC_SEXP1 = 6    # 2^-(eq8+ek8): descale fp8 score1 psum inside exp
DSC_SEXP2 = 7    # 2^-(eq28+ek8): descale fp8 score2 psum inside exp
NDSC = 8
EQ28 = 9         # fp8 scale exponent for q2 (|q2|max ~0.29 -> ~147)

_CACHE = {}


def build_kernel(collective=True):
    key = ("nc", collective)
    if key in _CACHE:
        return _CACHE[key]
    nc = bacc.Bacc("TRN2", target_bir_lowering=False, debug=False,
                   num_devices=N_CORES if collective else 1)

    # ---- I/O ----
    xT_d = nc.dram_tensor("xT", [D, S], BF16, kind="ExternalInput")
    xT8_d = nc.dram_tensor("xT8", [D, S], F8, kind="ExternalInput")
    wqk8_d = nc.dram_tensor("wqk8", [128, KTP * 2 * 512], F8, kind="ExternalInput")
    wv_d = nc.dram_tensor("wv", [D, CL], BF16, kind="ExternalInput")
    qkb_d = nc.dram_tensor("qkb", [4, 128], F32, kind="ExternalInput")
    vbb_d = nc.dram_tensor("vbb", [128, CL], F32, kind="ExternalInput")
    cw8_d = nc.dram_tensor("cw8", [128, 3 * KTP * 2 * CL], F8, kind="ExternalInput")
    cb_d = nc.dram_tensor("cb", [2, 128], F32, kind="ExternalInput")
    ow_d = nc.dram_tensor("ow", [CL, D], BF16, kind="ExternalInput")
    tri2_d = nc.dram_tensor("tri2", [128, 256], BF16, kind="ExternalInput")
    dsc_d = nc.dram_tensor("dsc", [128, NDSC], F32, kind="ExternalInput")
    outT_d = nc.dram_tensor("outT", [D, S], BF16, kind="ExternalOutput")

    xT_v = xT_d.ap().rearrange("(kt p) t -> p kt t", p=128)
    xT8_v = xT8_d.ap().rearrange("(ktp sub p) t -> p ktp sub t", p=128, sub=2)
    outT_v = outT_d.ap().rearrange("(m p) t -> p m t", p=128)

    with tile.TileContext(nc) as tc:
        with (
            tc.tile_pool(name="w", bufs=1) as wp,
            tc.tile_pool(name="big", bufs=1) as bigp,
            tc.tile_pool(name="xs", bufs=2) as xsp,
            tc.tile_pool(name="p", bufs=3) as pp,
            tc.tile_pool(name="nrm", bufs=2) as nrmp,
            tc.tile_pool(name="blk", bufs=2) as blkp,
            tc.tile_pool(name="ob", bufs=3) as obp,
            tc.tile_pool(name="ps", bufs=1, space="PSUM") as psp,
            tc.tile_pool(name="dram", bufs=1, space="DRAM") as dramp,
        ):
            # ---- load weights / constants ----
            # wqk8 + the first x8 block lead the DMA queue so the k
            # projection starts ASAP
            wqk8 = wp.tile([128, KTP, 2, 512], F8)
            nc.sync.dma_start(
                wqk8[:], wqk8_d.ap().rearrange("p (a s m) -> p a s m", a=KTP, s=2))
            xt8s = [wp.tile([128, KTP, 2, 512], F8, name=f"xt8_{j}")
                    for j in range(NJ)]
            nc.sync.dma_start(xt8s[0][:], xT8_v[:, :, :, ts(0, 512)])
            dsc = wp.tile([128, NDSC], F32)
            nc.sync.dma_start(dsc[:], dsc_d.ap())
            qkb = wp.tile([128, 4], F32)
            nc.sync.dma_start(qkb[:], qkb_d.ap().rearrange("m p -> p m"))
            wv = wp.tile([128, KT, CL], BF16)
            nc.sync.dma_start(wv[:], wv_d.ap().rearrange("(kt p) c -> p kt c", p=128))
            vbb = wp.tile([128, CL], F32)
            nc.sync.dma_start(vbb[:], vbb_d.ap())
            tri2 = wp.tile([128, 2, 128], BF16)
            nc.sync.dma_start(tri2[:], tri2_d.ap().rearrange("p (h t) -> p h t", h=2))
            tri8 = wp.tile([128, 128], F8)
            nc.vector.tensor_copy(out=tri8[:], in_=tri2[:, 0, :])
            # conv / out-proj weights are DMA'd during attention 1
            cw8 = wp.tile([128, 3, KTP, 2, CL], F8)
            cb = wp.tile([128, 2], F32)
            ow = wp.tile([128, 2, 8, 128], BF16)

            # ---- persistent activations ----
            # q/k/q2 in fp8 "dim-split" layout for double-row scores:
            # two tiles (heads 0,1 | heads 2,3), partition = (h%2)*32 + d
            # (d<32), sub 0 = dims 0:32, sub 1 = dims 32:64 (base partition
            # for matmul operands must be in {0,32,64})
            qd8 = [bigp.tile([64, 2, S], F8, name=f"qd8_{i}") for i in range(2)]
            kd8 = [bigp.tile([64, 2, S], F8, name=f"kd8_{i}") for i in range(2)]
            q2d8 = [bigp.tile([64, 2, S], F8, name=f"q2d8_{i}")
                    for i in range(2)]
            # v for ctx2 (bf16): cols 64:128 are ones -> den rows in psum
            v_sb = bigp.tile([128, NS, HPC, 128], BF16, name="v_sb")
            nc.vector.memset(v_sb[:, :, :, 64:128], 1.0)
            # v for ctx1 (fp8 double-row s-tile pairs): cols 64:128 are ONE8
            v8 = bigp.tile([128, NS // 2, 2, HPC, 128], F8, name="v8")
            nc.vector.memset(v8[:, :, :, :, 64:128], ONE8)
            ctxg = bigp.tile([128, KT, S + 2], F8, name="ctxg")
            nc.vector.memset(ctxg[:, :, 0:1], 0.0)
            nc.vector.memset(ctxg[:, :, S + 1:S + 2], 0.0)

            # 2-block gather payloads: [3,2] and [1,0] (descending order)
            cc_in = [dramp.tile([CL, 1024], F8, tag=f"ci{g}", name=f"ci{g}")
                     for g in range(2)]
            cc_out = [dramp.tile([D, 1024], F8, tag=f"co{g}", name=f"co{g}")
                      for g in range(2)]

            # ================= Phase A: K/V projections =================
            for j in range(NJ):
                if j > 0:
                    nc.sync.dma_start(xt8s[j][:], xT8_v[:, :, :, ts(j, 512)])
                xt = xsp.tile([128, KT, 512], BF16, tag="xt", bufs=2)
                nc.sync.dma_start(xt[:], xT_v[:, :, ts(j, 512)])
                # k (m=2,3) via fp8 double-row: m=2 -> dims 0:32 (sub 0),
                # m=3 -> dims 32:64 (sub 1), all 4 heads along partitions
                for m in range(2, 4):
                    ps = psp.tile([128, 512], F32, tag="mm", bufs=2)
                    for kp in range(KTP):
                        nc.tensor.matmul(ps[:], wqk8[:, kp, :, ts(m, 128)],
                                         xt8s[j][:, kp, :, :], perf_mode=DR,
                                         start=(kp == 0), stop=(kp == KTP - 1))
                    for i in range(2):
                        rowi = slice(64 * i, 64 * i + 64)
                        nc.vector.tensor_scalar(kd8[i][:, m % 2, ts(j, 512)],
                                                ps[rowi, :],
                                                dsc[rowi, DSC_K:DSC_K + 1],
                                                qkb[rowi, m:m + 1], MULT, ADD)
                # v token-major (bf16): [t, c] for the 4 s-tiles of this block
                for u in range(4):
                    ps = psp.tile([128, CL], F32, tag="mm", bufs=2)
                    for kt in range(KT):
                        nc.tensor.matmul(ps[:], xt[:, kt, ts(u, 128)],
                                         wv[:, kt, :],
                                         start=(kt == 0), stop=(kt == KT - 1))
                    st_i = 4 * j + u
                    nc.vector.tensor_tensor(
                        v_sb[:, st_i, :, 0:64],
                        ps.rearrange("p (h e) -> p h e", e=64),
                        vbb.rearrange("p (h e) -> p h e", e=64), ADD)
                # fp8 copy of v (scaled by V8S) for the ctx1 double-row
                nc.vector.tensor_scalar(
                    v8[:, 2 * j:2 * j + 2, :, :, 0:64].rearrange(
                        "p a b h d -> p (a b) h d"),
                    v_sb[:, 4 * j:4 * j + 4, :, 0:64],
                    dsc[:, DSC_V8:DSC_V8 + 1], None, MULT)

            # ============ pipelined attention 1 / gather / conv / attn 2 ====
            def qproj(j):
                for m in range(2):
                    ps = psp.tile([128, 512], F32, tag="mm", bufs=2)
                    for kp in range(KTP):
                        nc.tensor.matmul(ps[:], wqk8[:, kp, :, ts(m, 128)],
                                         xt8s[j][:, kp, :, :], perf_mode=DR,
                                         start=(kp == 0), stop=(kp == KTP - 1))
                    for i in range(2):
                        rowi = slice(64 * i, 64 * i + 64)
                        nc.vector.tensor_scalar(qd8[i][:, m, ts(j, 512)],
                                                ps[rowi, :],
                                                dsc[rowi, DSC_Q:DSC_Q + 1],
                                                qkb[rowi, m:m + 1], MULT, ADD)

            def attn1_head(j, h, blk, n_pairs):
                kp, row = h // 2, slice(64 * (h % 2), 64 * (h % 2) + 64)
                r32 = slice(32 * (h % 2), 32 * (h % 2) + 32)
                qt, kt8 = qd8[h // 2], kd8[h // 2]
                cps = psp.tile([128, 512], F32, tag="ctx", bufs=2, name="ctx1")
                pend = None

                def expctx1(stp, pr, c0, c1v, diag):
                    p8t = pp.tile([128, 2, 512], F8, tag="p1")
                    nc.scalar.activation(p8t[:, :, c0:512],
                                         stp[:, :, c0:512], EXP,
                                         bias=dsc[:, DSC_LNP8:DSC_LNP8 + 1],
                                         scale=dsc[:, DSC_SEXP1:DSC_SEXP1 + 1])
                    if diag:
                        # zero the below-diagonal strip of subtile 1,
                        # tri-mask both subtiles' diagonal strips
                        nc.gpsimd.memset(p8t[:, 1, c0:c1v], 0.0)
                        nc.vector.tensor_tensor(
                            p8t[:, 0, c0:c0 + 128], p8t[:, 0, c0:c0 + 128],
                            tri8[:], MULT)
                        nc.vector.tensor_tensor(
                            p8t[:, 1, c1v:c1v + 128], p8t[:, 1, c1v:c1v + 128],
                            tri8[:], MULT)
                    nc.tensor.matmul(cps[:, c0:512], v8[:, pr, :, h, :],
                                     p8t[:, :, c0:512], perf_mode=DR,
                                     start=(pr == 0), stop=(pr == n_pairs - 1))

                for pr in range(n_pairs):
                    i0, i1 = 2 * pr, 2 * pr + 1
                    rr = i0 - 4 * j
                    c0 = 128 * rr if rr > 0 else 0
                    c1v = c0 + 128 if rr >= 0 else 0
                    stp = psp.tile([128, 2, 512], F32, tag="st", bufs=2)
                    nc.tensor.matmul(stp[:, 0, c0:512],
                                     kt8[r32, :, ts(i0, 128)],
                                     qt[r32, :, j * 512 + c0:(j + 1) * 512],
                                     perf_mode=DR)
                    nc.tensor.matmul(stp[:, 1, c1v:512],
                                     kt8[r32, :, ts(i1, 128)],
                                     qt[r32, :, j * 512 + c1v:(j + 1) * 512],
                                     perf_mode=DR)
                    if pend is not None:
                        expctx1(*pend)
                    pend = (stp, pr, c0, c1v, rr >= 0)
                expctx1(*pend)
                # normalize: reciprocal of den rows (64:128), multiply the
                # ctx rows -> fp8 block (scale CTXGS)
                dsb = nrmp.tile([64, 512], F32, tag="dsb")
                nc.vector.tensor_copy(out=dsb[:], in_=cps[64:128, :])
                rc = nrmp.tile([64, 512], F32, tag="rc")
                nc.vector.reciprocal_approx_fast(rc[:], dsb[:])
                nc.vector.tensor_tensor(blk[row, kp, :], cps[0:64, :],
                                        rc[:], MULT)

            def attn1_block(j, interleave=()):
                """interleave: callables run between head iterations (fill
                the PE's exp-dependency gaps with independent matmul work)."""
                blk = blkp.tile([128, 2, 512], F8, tag="c1")
                inter = list(interleave)
                for h in range(HPC):
                    attn1_head(j, h, blk, 2 * j + 2)
                    if inter:
                        inter.pop(0)()
                g, half = (0, 0 if j == 3 else 1) if j >= 2 else \
                          (1, 0 if j == 1 else 1)
                nc.sync.dma_start(
                    cc_in[g].opt()[:, ts(half, 512)].rearrange(
                        "(k p) t -> p k t", p=128),
                    blk[:])
                return g

            def gather(g, jhi, jlo):
                if collective:
                    nc.gpsimd.collective_compute(
                        "AllGather", mybir.AluOpType.bypass,
                        replica_groups=GROUPS,
                        ins=[cc_in[g].opt()], outs=[cc_out[g].opt()])
                else:
                    for g4 in range(4):
                        nc.sync.dma_start(
                            cc_out[g].opt()[CL * g4:CL * (g4 + 1), :],
                            cc_in[g].opt()[:])
                for half, j in ((0, jhi), (1, jlo)):
                    nc.sync.dma_start(
                        ctxg[:, :, 1 + j * 512:1 + (j + 1) * 512],
                        cc_out[g].opt()[:, ts(half, 512)].rearrange(
                            "(kt p) t -> p kt t", p=128))

            def conv_chain(j, ot):
                ps = psp.tile([128, 512], F32, tag="mm", bufs=2)
                first = True
                for tap in range(3):
                    for kp in range(KTP):
                        nc.tensor.matmul(
                            ps[:], cw8[:, tap, kp, :, ts(ot, 128)],
                            ctxg[:, 2 * kp:2 * kp + 2,
                                 j * 512 + tap:j * 512 + tap + 512],
                            perf_mode=DR, start=first,
                            stop=(tap == 2 and kp == KTP - 1))
                        first = False
                for i in range(2):
                    rowi = slice(64 * i, 64 * i + 64)
                    nc.vector.tensor_scalar(q2d8[i][:, ot, ts(j, 512)],
                                            ps[rowi, :],
                                            dsc[rowi, DSC_CV:DSC_CV + 1],
                                            cb[rowi, ot:ot + 1], MULT, ADD)

            def attn2_block(j, interleave=()):
                blk2 = blkp.tile([128, 2, 512], BF16, tag="c2")
                i_last = 4 * j + 3
                inter = list(interleave)
                for kp in range(2):
                    cps2 = [psp.tile([128, 512], F32, tag="ctx", bufs=2,
                                     name="ctx2") for _ in range(2)]
                    pend = None

                    def expctx2(st_v, i, c0, cps2=cps2, kp=kp):
                        p = pp.tile([128, 2, 512], BF16, tag="p2")
                        nc.scalar.activation(p[:, :, c0:512],
                                             st_v[:, :, c0:512], EXP,
                                             bias=dsc[:, DSC_ZERO:DSC_ZERO + 1],
                                             scale=dsc[:, DSC_SEXP2:DSC_SEXP2 + 1])
                        if i - 4 * j >= 0:
                            nc.vector.tensor_tensor(p[:, :, c0:c0 + 128],
                                                    p[:, :, c0:c0 + 128],
                                                    tri2[:], MULT)
                        for hh in range(2):
                            nc.tensor.matmul(cps2[hh][:, c0:512],
                                             v_sb[:, i, 2 * kp + hh, :],
                                             p[:, hh, c0:512],
                                             start=(i == 0), stop=(i == i_last))

                    for i in range(4 * j + 4):
                        r = i - 4 * j
                        c0 = 128 * r if r > 0 else 0
                        st = psp.tile([128, 2, 512], F32, tag="st", bufs=2)
                        for hh in range(2):
                            r32 = slice(32 * hh, 32 * hh + 32)
                            nc.tensor.matmul(st[:, hh, c0:512],
                                             kd8[kp][r32, :, ts(i, 128)],
                                             q2d8[kp][r32, :,
                                                      j * 512 + c0:(j + 1) * 512],
                                             perf_mode=DR)
                        if pend is not None:
                            expctx2(*pend)
                        pend = (st, i, c0)
                    expctx2(*pend)
                    for hh in range(2):
                        dsb = nrmp.tile([64, 512], F32, tag="dsb")
                        nc.vector.tensor_copy(out=dsb[:], in_=cps2[hh][64:128, :])
                        rc = nrmp.tile([64, 512], F32, tag="rc")
                        nc.vector.reciprocal_approx_fast(rc[:], dsb[:])
                        nc.vector.tensor_tensor(blk2[64 * hh:64 * hh + 64, kp, :],
                                                cps2[hh][0:64, :], rc[:], MULT)
                    if inter:
                        inter.pop(0)()
                return blk2

            def outproj(blk2, j, half=None):
                ms = range(8) if half is None else range(4 * half, 4 * half + 4)
                for m in ms:
                    ps = psp.tile([128, 512], F32, tag="mm", bufs=2)
                    for kt in range(2):
                        nc.tensor.matmul(ps[:], ow[:, kt, m, :],
                                         blk2[:, kt, :],
                                         start=(kt == 0), stop=(kt == 1))
                    ob = obp.tile([128, 512], BF16, tag="ob")
                    nc.vector.tensor_copy(out=ob[:], in_=ps[:])
                    nc.sync.dma_start(outT_v[:, m, ts(j, 512)], ob[:])

            # schedule: big attn-1 blocks first; 2-block gathers issue early;
            # conv(3) fills attn-1's tail blocks; out-proj trails one block
            qproj(3)
            attn1_block(3)
            qproj(2)
            attn1_block(2)
            gather(0, 3, 2)
            # conv/out-proj weights land during attention-1 compute
            nc.sync.dma_start(
                cw8[:], cw8_d.ap().rearrange(
                    "p (a b s o) -> p a b s o", a=3, b=KTP, s=2))
            nc.sync.dma_start(cb[:], cb_d.ap().rearrange("m p -> p m"))
            nc.sync.dma_start(
                ow[:], ow_d.ap().rearrange(
                    "(kt p) (m q) -> p kt m q", p=128, q=128))
            qproj(1)
            attn1_block(1)
            qproj(0)
            attn1_block(0, interleave=(lambda: conv_chain(3, 0),
                                       lambda: conv_chain(3, 1)))
            gather(1, 1, 0)
            blk2_3 = attn2_block(3)
            conv_chain(2, 0)
            conv_chain(2, 1)
            blk2_2 = attn2_block(2, interleave=(lambda: outproj(blk2_3, 3, 0),
                                                lambda: outproj(blk2_3, 3, 1)))
            conv_chain(1, 0)
            conv_chain(1, 1)
            blk2_1 = attn2_block(1, interleave=(lambda: outproj(blk2_2, 2, 0),
                                                lambda: outproj(blk2_2, 2, 1)))
            conv_chain(0, 0)
            conv_chain(0, 1)
            blk2_0 = attn2_block(0, interleave=(lambda: outproj(blk2_1, 1, 0),
                                                lambda: outproj(blk2_1, 1, 1)))
            outproj(blk2_0, 0)

    nc.compile()
    _CACHE[key] = nc
    return nc


def _pow2_scale(arr, target=224.0):
    m = float(np.max(np.abs(arr)))
    if m <= 0:
        return 0
    return int(math.floor(math.log2(target / m)))


def prep_inputs(x, Wqkv_w, Wqkv_b, conv_w, conv_b, out_w, out_b):
    """Build the 8 per-core input maps from the full problem inputs."""
    x = np.asarray(x, np.float32)
    Wqkv_w = np.asarray(Wqkv_w, np.float32)
    Wqkv_b = np.asarray(Wqkv_b, np.float32)
    conv_w = np.asarray(conv_w, np.float32)
    conv_b = np.asarray(conv_b, np.float32)
    out_w = np.asarray(out_w, np.float32)

    scale = 1.0 / np.sqrt(DH).astype(np.float32)
    tri = (np.arange(128)[None, :] >= np.arange(128)[:, None]).astype(np.float32)
    tri2 = np.concatenate([tri, tri], axis=1).astype(BFNP)

    ex = [_pow2_scale(x[b]) for b in range(B)]

    # dim-split column order: within an m-tile, col c = h*32 + d covers
    # head h0+h, dims d (+32 for the "B" tile)
    perm = (np.arange(HPC)[:, None] * DH + np.arange(32)[None, :]).ravel()

    in_maps = []
    for g in range(N_CORES):
        b, hg = g // 4, g % 4
        h0 = HPC * hg
        # m-tiles: [qA (dims 0:32), qB (32:64), kA, kB], heads along cols
        qrows = Wqkv_w[h0 * DH:(h0 + HPC) * DH, :] * scale   # [256, D]
        qbias = Wqkv_b[h0 * DH:(h0 + HPC) * DH] * scale
        krows = Wqkv_w[D + h0 * DH:D + (h0 + HPC) * DH, :]
        kbias = Wqkv_b[D + h0 * DH:D + (h0 + HPC) * DH]
        rows = [qrows[perm], qrows[perm + 32], krows[perm], krows[perm + 32]]
        # fp8 output scales for q/k (device-side activations)
        qv = x[b] @ qrows.T
        kv = x[b] @ krows.T
        eq8 = _pow2_scale(qv, 128.0)
        ek8 = _pow2_scale(kv, 128.0)
        biases = [qbias[perm] * 2.0 ** eq8, qbias[perm + 32] * 2.0 ** eq8,
                  kbias[perm] * 2.0 ** ek8, kbias[perm + 32] * 2.0 ** ek8]
        wqk = np.concatenate(rows, axis=0)  # [512 ch, D]
        eq = _pow2_scale(wqk[0:256])
        ek = _pow2_scale(wqk[256:512])
        wqk_s = wqk * np.concatenate([np.full(256, 2.0 ** eq, np.float32),
                                      np.full(256, 2.0 ** ek, np.float32)])[:, None]
        # [512, D] -> [D, 512] -> [KTP, 2, 128, 512] -> [128, KTP, 2, 512]
        wqk8 = np.ascontiguousarray(
            wqk_s.T.reshape(KTP, 2, 128, 512).transpose(2, 0, 1, 3)
        ).astype(E4NP).reshape(128, KTP * 2 * 512)
        qkb = np.stack(biases, axis=1).astype(np.float32)  # [128, 4] -> [4,128]?
        qkb = np.ascontiguousarray(qkb.T)  # [4, 128]
        c0 = CL * hg
        wv = np.ascontiguousarray(
            Wqkv_w[2 * D + c0:2 * D + c0 + CL, :].T).astype(BFNP)
        vbb = np.ascontiguousarray(
            np.broadcast_to(Wqkv_b[2 * D + c0:2 * D + c0 + CL], (128, CL)))
        # conv weights: [o, i, tap] -> fp8 [128p, tap, ktp, sub, o'] with the
        # same dim-split order on output columns (o' tiles: A=dims0:32, B)
        cwl = conv_w[c0:c0 + CL, :, :] * scale
        ecw = _pow2_scale(cwl)
        operm = np.concatenate([perm, perm + 32])  # [256] output order A|B
        cw8 = np.ascontiguousarray(
            (cwl[operm] * 2.0 ** ecw).transpose(2, 1, 0)   # [tap, i, o']
            .reshape(3, KTP, 2, 128, CL).transpose(3, 0, 1, 2, 4)
        ).astype(E4NP).reshape(128, 3 * KTP * 2 * CL)
        cb = (conv_b[c0:c0 + CL][operm] * scale * 2.0 ** EQ28
              ).reshape(2, 128).astype(np.float32)
        owm = np.ascontiguousarray(
            out_w[:, c0:c0 + CL].T).astype(BFNP)  # [CL, D]
        dsc_row = np.zeros(NDSC, np.float32)
        dsc_row[DSC_Q] = 2.0 ** (-(ex[b] + eq) + eq8)
        dsc_row[DSC_K] = 2.0 ** (-(ex[b] + ek) + ek8)
        dsc_row[DSC_CV] = 2.0 ** (-(ecw + int(math.log2(CTXGS))) + EQ28)
        dsc_row[DSC_LNP8] = math.log(P8S)
        dsc_row[DSC_ZERO] = 0.0
        dsc_row[DSC_V8] = V8S
        dsc_row[DSC_SEXP1] = 2.0 ** (-(eq8 + ek8))
        dsc_row[DSC_SEXP2] = 2.0 ** (-(EQ28 + ek8))
        dsc = np.ascontiguousarray(np.broadcast_to(dsc_row, (128, NDSC)))
        in_maps.append({
            "xT": np.ascontiguousarray(x[b].T).astype(BFNP),
            "xT8": np.ascontiguousarray(x[b].T * 2.0 ** ex[b]).astype(E4NP),
            "wqk8": wqk8, "wv": wv,
            "qkb": np.ascontiguousarray(qkb),
            "vbb": vbb, "cw8": cw8,
            "cb": np.ascontiguousarray(cb),
            "ow": owm, "tri2": tri2,
            "dsc": dsc,
        })
    return in_maps


def postprocess(results, out_b):
    out_b = np.asarray(out_b, np.float32)
    out = np.empty((B, S, D), np.float32)
    for b in range(B):
        acc = np.zeros((D, S), np.float64)
        for g in GROUPS[b]:
            acc += np.asarray(results[g]["outT"], np.float64)
        out[b] = acc.T.astype(np.float32) + out_b[None, :]
    return out


def kernel(x, Wqkv_w, Wqkv_b, conv_w, conv_b, out_w, out_b):
    nc = build_kernel()
    in_maps = prep_inputs(x, Wqkv_w, Wqkv_b, conv_w, conv_b, out_w, out_b)
    res = run_bass_kernel_spmd(nc, in_maps, core_ids=list(range(N_CORES)))
    return postprocess(res.results, out_b)


# revision 38
# speedup vs baseline: 1.0965x; 1.0965x over previous
"""Trainium2 Bass kernel for nn_MHC (dense transformer block: QKV -> causal
attention -> conv1d(k=3) -> causal attention (same K/V) -> out proj).

Sharding over 8 NeuronCores: data-parallel on batch (2) x tensor-parallel on
heads (16 heads -> 4 per core). Cores 0-3 own batch 0, cores 4-7 batch 1.
Per-token-block AllGather (groups of 4) exchanges attention-1 context (fp8
payload) so each core can run the channel-mixing conv for its own output
channels; gathers issue immediately after each block's normalize and overlap
the next block's attention compute.

fp8 (e4m3) DoubleRow matmuls carry the q/k projections, the conv1d, and the
attention-1 context accumulation (2-4x bf16 PE throughput); scores, v, ctx2
and the out projection stay bf16 for accuracy. Softmax denominators ride as
64 broadcast "ones" rows in each ctx matmul's stationary operand, so
normalization is a PSUM-direct copy/reciprocal/multiply on the vector engine
and the scalar engine runs exp only.
"""

import math

import numpy as np
import ml_dtypes

import concourse.bacc as bacc
import concourse.mybir as mybir
import concourse.tile as tile
from concourse.bass import ts
from concourse.bass_utils import run_bass_kernel_spmd

# Problem shapes (hardcoded per contract)
B, S, D = 2, 2048, 1024
H, DH = 16, 64
N_CORES = 8
HPC = 4          # heads per core
CL = HPC * DH    # 256 local channels
KT = D // 128    # 8 k-tiles over the model dim
KTP = KT // 2    # 4 fp8 double-row k-tile pairs
NJ = S // 512    # 4 t-blocks of 512
NS = S // 128    # 16 s-tiles of 128
GROUPS = [[0, 1, 2, 3], [4, 5, 6, 7]]

F32 = mybir.dt.float32
BF16 = mybir.dt.bfloat16
F8 = mybir.dt.float8e4
EXP = mybir.ActivationFunctionType.Exp
MULT = mybir.AluOpType.mult
ADD = mybir.AluOpType.add
DR = mybir.MatmulPerfMode.DoubleRow

E4NP = ml_dtypes.float8_e4m3
BFNP = ml_dtypes.bfloat16

P8S = 2.0        # fp8 scale on exp(score) in attention 1 (headroom for the
                 # below-diagonal scores that tri masks AFTER exp: fp8
                 # overflow there would turn the masked zeros into NaN)
V8S = 32.0       # fp8 scale on v for the attention-1 ctx matmul
ONE8 = 0.5       # ones-column value in v8
CTXGS = 64.0     # scale of the gathered fp8 ctx: P8S*V8S / (P8S*ONE8)
# ctx1 psum = (8p)(32v) = 256*sum(pv); den rows = (8p)(0.5) = 4*sum(p);
# evacuate-multiply by 1/denrows -> 64 * ctx = CTXGS * ctx.

# dsc columns (per-core dynamic constants, broadcast to 128 partitions)
DSC_Q = 0        # 2^-(ex+eq)*2^eq8: q-proj psum -> fp8 q
DSC_K = 1        # 2^-(ex+ek)*2^ek8: k-proj psum -> fp8 k
DSC_CV = 2       # 2^-(ecw+6)*2^eq28: conv psum -> fp8 q2
DSC_LNP8 = 3     # ln(P8S): exp bias for attention 1
DSC_ZERO = 4     # 0.0: exp bias for attention 2
DSC_V8 = 5       # V8S
DSC_SEXP1 = 6    # 2^-(eq8+ek8): descale fp8 score1 psum inside exp
DSC_SEXP2 = 7    # 2^-(eq28+ek8): descale fp8 score2 psum inside exp
NDSC = 8
EQ28 = 9         # fp8 scale exponent for q2 (|q2|max ~0.29 -> ~147)

_CACHE = {}


def build_kernel(collective=True):
    key = ("nc", collective)
    if key in _CACHE:
        return _CACHE[key]
    nc = bacc.Bacc("TRN2", target_bir_lowering=False, debug=False,
                   num_devices=N_CORES if collective else 1)

    # ---- I/O ----
    xT_d = nc.dram_tensor("xT", [D, S], BF16, kind="ExternalInput")
    xT8_d = nc.dram_tensor("xT8", [D, S], F8, kind="ExternalInput")
    wqk8_d = nc.dram_tensor("wqk8", [128, KTP * 2 * 512], F8, kind="ExternalInput")
    wv_d = nc.dram_tensor("wv", [D, CL], BF16, kind="ExternalInput")
    qkb_d = nc.dram_tensor("qkb", [4, 128], F32, kind="ExternalInput")
    vbb_d = nc.dram_tensor("vbb", [128, CL], F32, kind="ExternalInput")
    cw8_d = nc.dram_tensor("cw8", [128, 3 * KTP * 2 * CL], F8, kind="ExternalInput")
    cb_d = nc.dram_tensor("cb", [2, 128], F32, kind="ExternalInput")
    ow_d = nc.dram_tensor("ow", [CL, D], BF16, kind="ExternalInput")
    tri2_d = nc.dram_tensor("tri2", [128, 256], BF16, kind="ExternalInput")
    dsc_d = nc.dram_tensor("dsc", [128, NDSC], F32, kind="ExternalInput")
    outT_d = nc.dram_tensor("outT", [D, S], BF16, kind="ExternalOutput")

    xT_v = xT_d.ap().rearrange("(kt p) t -> p kt t", p=128)
    xT8_v = xT8_d.ap().rearrange("(ktp sub p) t -> p ktp sub t", p=128, sub=2)
    outT_v = outT_d.ap().rearrange("(m p) t -> p m t", p=128)

    with tile.TileContext(nc) as tc:
        with (
            tc.tile_pool(name="w", bufs=1) as wp,
            tc.tile_pool(name="big", bufs=1) as bigp,
            tc.tile_pool(name="xs", bufs=2) as xsp,
            tc.tile_pool(name="p", bufs=3) as pp,
            tc.tile_pool(name="nrm", bufs=2) as nrmp,
            tc.tile_pool(name="blk", bufs=2) as blkp,
            tc.tile_pool(name="ob", bufs=3) as obp,
            tc.tile_pool(name="ps", bufs=1, space="PSUM") as psp,
            tc.tile_pool(name="dram", bufs=1, space="DRAM") as dramp,
        ):
            # ---- load weights / constants ----
            # wqk8 + the first x8 block lead the DMA queue so the k
            # projection starts ASAP
            wqk8 = wp.tile([128, KTP, 2, 512], F8)
            nc.sync.dma_start(
                wqk8[:], wqk8_d.ap().rearrange("p (a s m) -> p a s m", a=KTP, s=2))
            xt8s = [wp.tile([128, KTP, 2, 512], F8, name=f"xt8_{j}")
                    for j in range(NJ)]
            nc.sync.dma_start(xt8s[0][:], xT8_v[:, :, :, ts(0, 512)])
            dsc = wp.tile([128, NDSC], F32)
            nc.sync.dma_start(dsc[:], dsc_d.ap())
            qkb = wp.tile([128, 4], F32)
            nc.sync.dma_start(qkb[:], qkb_d.ap().rearrange("m p -> p m"))
            wv = wp.tile([128, KT, CL], BF16)
            nc.sync.dma_start(wv[:], wv_d.ap().rearrange("(kt p) c -> p kt c", p=128))
            vbb = wp.tile([128, CL], F32)
            nc.sync.dma_start(vbb[:], vbb_d.ap())
            tri2 = wp.tile([128, 2, 128], BF16)
            nc.sync.dma_start(tri2[:], tri2_d.ap().rearrange("p (h t) -> p h t", h=2))
            tri8 = wp.tile([128, 128], F8)
            nc.vector.tensor_copy(out=tri8[:], in_=tri2[:, 0, :])
            # conv / out-proj weights are DMA'd during attention 1
            cw8 = wp.tile([128, 3, KTP, 2, CL], F8)
            cb = wp.tile([128, 2], F32)
            ow = wp.tile([128, 2, 8, 128], BF16)

            # ---- persistent activations ----
            qpair = bigp.tile([128, 2, S], BF16, name="qpair")
            kpair = bigp.tile([128, 2, S], BF16, name="kpair")
            q2pair = bigp.tile([128, 2, S], BF16, name="q2pair")
            # v for ctx2 (bf16): cols 64:128 are ones -> den rows in psum
            v_sb = bigp.tile([128, NS, HPC, 128], BF16, name="v_sb")
            nc.vector.memset(v_sb[:, :, :, 64:128], 1.0)
            # v for ctx1 (fp8 double-row s-tile pairs): cols 64:128 are ONE8
            v8 = bigp.tile([128, NS // 2, 2, HPC, 128], F8, name="v8")
            nc.vector.memset(v8[:, :, :, :, 64:128], ONE8)
            ctxg = bigp.tile([128, KT, S + 2], F8, name="ctxg")
            nc.vector.memset(ctxg[:, :, 0:1], 0.0)
            nc.vector.memset(ctxg[:, :, S + 1:S + 2], 0.0)

            # 2-block gather payloads: [3,2] and [1,0] (descending order)
            cc_in = [dramp.tile([CL, 1024], F8, tag=f"ci{g}", name=f"ci{g}")
                     for g in range(2)]
            cc_out = [dramp.tile([D, 1024], F8, tag=f"co{g}", name=f"co{g}")
                      for g in range(2)]

            # ================= Phase A: K/V projections =================
            for j in range(NJ):
                if j > 0:
                    nc.sync.dma_start(xt8s[j][:], xT8_v[:, :, :, ts(j, 512)])
                xt = xsp.tile([128, KT, 512], BF16, tag="xt", bufs=2)
                nc.sync.dma_start(xt[:], xT_v[:, :, ts(j, 512)])
                # k (m=2,3) via fp8 double-row: m=2 -> dims 0:32 (sub 0),
                # m=3 -> dims 32:64 (sub 1), all 4 heads along partitions
                for m in range(2, 4):
                    ps = psp.tile([128, 512], F32, tag="mm", bufs=2)
                    for kp in range(KTP):
                        nc.tensor.matmul(ps[:], wqk8[:, kp, :, ts(m, 128)],
                                         xt8s[j][:, kp, :, :], perf_mode=DR,
                                         start=(kp == 0), stop=(kp == KTP - 1))
                    nc.vector.tensor_scalar(kpair[:, m % 2, ts(j, 512)], ps[:],
                                            dsc[:, DSC_K:DSC_K + 1],
                                            qkb[:, m:m + 1], MULT, ADD)
                # v token-major (bf16): [t, c] for the 4 s-tiles of this block
                for u in range(4):
                    ps = psp.tile([128, CL], F32, tag="mm", bufs=2)
                    for kt in range(KT):
                        nc.tensor.matmul(ps[:], xt[:, kt, ts(u, 128)],
                                         wv[:, kt, :],
                                         start=(kt == 0), stop=(kt == KT - 1))
                    st_i = 4 * j + u
                    nc.vector.tensor_tensor(
                        v_sb[:, st_i, :, 0:64],
                        ps.rearrange("p (h e) -> p h e", e=64),
                        vbb.rearrange("p (h e) -> p h e", e=64), ADD)
                # fp8 copy of v (scaled by V8S) for the ctx1 double-row
                nc.vector.tensor_scalar(
                    v8[:, 2 * j:2 * j + 2, :, :, 0:64].rearrange(
                        "p a b h d -> p (a b) h d"),
                    v_sb[:, 4 * j:4 * j + 4, :, 0:64],
                    dsc[:, DSC_V8:DSC_V8 + 1], None, MULT)

            # ============ pipelined attention 1 / gather / conv / attn 2 ====
            def qproj(j):
                for m in range(2):
                    ps = psp.tile([128, 512], F32, tag="mm", bufs=2)
                    for kp in range(KTP):
                        nc.tensor.matmul(ps[:], wqk8[:, kp, :, ts(m, 128)],
                                         xt8s[j][:, kp, :, :], perf_mode=DR,
                                         start=(kp == 0), stop=(kp == KTP - 1))
                    nc.vector.tensor_scalar(qpair[:, m, ts(j, 512)], ps[:],
                                            dsc[:, DSC_Q:DSC_Q + 1],
                                            qkb[:, m:m + 1], MULT, ADD)

            def attn1_head(j, h, blk, n_pairs):
                kp, row = h // 2, slice(64 * (h % 2), 64 * (h % 2) + 64)
                cps = psp.tile([128, 512], F32, tag="ctx", bufs=2, name="ctx1")
                pend = None

                def expctx1(stp, pr, c0, c1v, diag):
                    p8t = pp.tile([128, 2, 512], F8, tag="p1")
                    nc.scalar.activation(p8t[:, :, c0:512],
                                         stp[:, :, c0:512], EXP,
                                         bias=dsc[:, DSC_LNP8:DSC_LNP8 + 1])
                    if diag:
                        # zero the below-diagonal strip of subtile 1,
                        # tri-mask both subtiles' diagonal strips
                        nc.gpsimd.memset(p8t[:, 1, c0:c1v], 0.0)
                        nc.vector.tensor_tensor(
                            p8t[:, 0, c0:c0 + 128], p8t[:, 0, c0:c0 + 128],
                            tri8[:], MULT)
                        nc.vector.tensor_tensor(
                            p8t[:, 1, c1v:c1v + 128], p8t[:, 1, c1v:c1v + 128],
                            tri8[:], MULT)
                    nc.tensor.matmul(cps[:, c0:512], v8[:, pr, :, h, :],
                                     p8t[:, :, c0:512], perf_mode=DR,
                                     start=(pr == 0), stop=(pr == n_pairs - 1))

                for pr in range(n_pairs):
                    i0, i1 = 2 * pr, 2 * pr + 1
                    rr = i0 - 4 * j
                    c0 = 128 * rr if rr > 0 else 0
                    c1v = c0 + 128 if rr >= 0 else 0
                    stp = psp.tile([128, 2, 512], F32, tag="st", bufs=2)
                    nc.tensor.matmul(stp[:, 0, c0:512],
                                     kpair[row, kp, ts(i0, 128)],
                                     qpair[row, kp, j * 512 + c0:(j + 1) * 512])
                    nc.tensor.matmul(stp[:, 1, c1v:512],
                                     kpair[row, kp, ts(i1, 128)],
                                     qpair[row, kp, j * 512 + c1v:(j + 1) * 512])
                    if pend is not None:
                        expctx1(*pend)
                    pend = (stp, pr, c0, c1v, rr >= 0)
                expctx1(*pend)
                # normalize: reciprocal of den rows (64:128), multiply the
                # ctx rows -> fp8 block (scale CTXGS)
                dsb = nrmp.tile([64, 512], F32, tag="dsb")
                nc.vector.tensor_copy(out=dsb[:], in_=cps[64:128, :])
                rc = nrmp.tile([64, 512], F32, tag="rc")
                nc.vector.reciprocal_approx_fast(rc[:], dsb[:])
                nc.vector.tensor_tensor(blk[row, kp, :], cps[0:64, :],
                                        rc[:], MULT)

            def attn1_block(j, interleave=()):
                """interleave: callables run between head iterations (fill
                the PE's exp-dependency gaps with independent matmul work)."""
                blk = blkp.tile([128, 2, 512], F8, tag="c1")
                inter = list(interleave)
                for h in range(HPC):
                    attn1_head(j, h, blk, 2 * j + 2)
                    if inter:
                        inter.pop(0)()
                g, half = (0, 0 if j == 3 else 1) if j >= 2 else \
                          (1, 0 if j == 1 else 1)
                nc.sync.dma_start(
                    cc_in[g].opt()[:, ts(half, 512)].rearrange(
                        "(k p) t -> p k t", p=128),
                    blk[:])
                return g

            def gather(g, jhi, jlo):
                if collective:
                    nc.gpsimd.collective_compute(
                        "AllGather", mybir.AluOpType.bypass,
                        replica_groups=GROUPS,
                        ins=[cc_in[g].opt()], outs=[cc_out[g].opt()])
                else:
                    for g4 in range(4):
                        nc.sync.dma_start(
                            cc_out[g].opt()[CL * g4:CL * (g4 + 1), :],
                            cc_in[g].opt()[:])
                for half, j in ((0, jhi), (1, jlo)):
                    nc.sync.dma_start(
                        ctxg[:, :, 1 + j * 512:1 + (j + 1) * 512],
                        cc_out[g].opt()[:, ts(half, 512)].rearrange(
                            "(kt p) t -> p kt t", p=128))

            def conv_chain(j, ot):
                ps = psp.tile([128, 512], F32, tag="mm", bufs=2)
                first = True
                for tap in range(3):
                    for kp in range(KTP):
                        nc.tensor.matmul(
                            ps[:], cw8[:, tap, kp, :, ts(ot, 128)],
                            ctxg[:, 2 * kp:2 * kp + 2,
                                 j * 512 + tap:j * 512 + tap + 512],
                            perf_mode=DR, start=first,
                            stop=(tap == 2 and kp == KTP - 1))
                        first = False
                nc.vector.tensor_scalar(q2pair[:, ot, ts(j, 512)], ps[:],
                                        dsc[:, DSC_CV:DSC_CV + 1],
                                        cb[:, ot:ot + 1], MULT, ADD)

            def attn2_block(j, interleave=()):
                blk2 = blkp.tile([128, 2, 512], BF16, tag="c2")
                i_last = 4 * j + 3
                inter = list(interleave)
                for kp in range(2):
                    cps2 = [psp.tile([128, 512], F32, tag="ctx", bufs=2,
                                     name="ctx2") for _ in range(2)]
                    pend = None

                    def expctx2(st_v, i, c0, cps2=cps2, kp=kp):
                        p = pp.tile([128, 2, 512], BF16, tag="p2")
                        nc.scalar.activation(p[:, :, c0:512],
                                             st_v[:, :, c0:512], EXP,
                                             bias=dsc[:, DSC_ZERO:DSC_ZERO + 1])
                        if i - 4 * j >= 0:
                            nc.vector.tensor_tensor(p[:, :, c0:c0 + 128],
                                                    p[:, :, c0:c0 + 128],
                                                    tri2[:], MULT)
                        for hh in range(2):
                            nc.tensor.matmul(cps2[hh][:, c0:512],
                                             v_sb[:, i, 2 * kp + hh, :],
                                             p[:, hh, c0:512],
                                             start=(i == 0), stop=(i == i_last))

                    for i in range(4 * j + 4):
                        r = i - 4 * j
                        c0 = 128 * r if r > 0 else 0
                        st = psp.tile([128, 2, 512], F32, tag="st", bufs=2)
                        for hh in range(2):
                            rowh = slice(64 * hh, 64 * hh + 64)
                            nc.tensor.matmul(st[:, hh, c0:512],
                                             kpair[rowh, kp, ts(i, 128)],
                                             q2pair[rowh, kp,
                                                    j * 512 + c0:(j + 1) * 512])
                        if pend is not None:
                            expctx2(*pend)
                        pend = (st, i, c0)
                    expctx2(*pend)
                    for hh in range(2):
                        dsb = nrmp.tile([64, 512], F32, tag="dsb")
                        nc.vector.tensor_copy(out=dsb[:], in_=cps2[hh][64:128, :])
                        rc = nrmp.tile([64, 512], F32, tag="rc")
                        nc.vector.reciprocal_approx_fast(rc[:], dsb[:])
                        nc.vector.tensor_tensor(blk2[64 * hh:64 * hh + 64, kp, :],
                                                cps2[hh][0:64, :], rc[:], MULT)
                    if inter:
                        inter.pop(0)()
                return blk2

            def outproj(blk2, j, half=None):
                ms = range(8) if half is None else range(4 * half, 4 * half + 4)
                for m in ms:
                    ps = psp.tile([128, 512], F32, tag="mm", bufs=2)
                    for kt in range(2):
                        nc.tensor.matmul(ps[:], ow[:, kt, m, :],
                                         blk2[:, kt, :],
                                         start=(kt == 0), stop=(kt == 1))
                    ob = obp.tile([128, 512], BF16, tag="ob")
                    nc.vector.tensor_copy(out=ob[:], in_=ps[:])
                    nc.sync.dma_start(outT_v[:, m, ts(j, 512)], ob[:])

            # schedule: big attn-1 blocks first; 2-block gathers issue early;
            # conv(3) fills attn-1's tail blocks; out-proj trails one block
            qproj(3)
            attn1_block(3)
            qproj(2)
            attn1_block(2)
            gather(0, 3, 2)
            # conv/out-proj weights land during attention-1 compute
            nc.sync.dma_start(
                cw8[:], cw8_d.ap().rearrange(
                    "p (a b s o) -> p a b s o", a=3, b=KTP, s=2))
            nc.sync.dma_start(cb[:], cb_d.ap().rearrange("m p -> p m"))
            nc.sync.dma_start(
                ow[:], ow_d.ap().rearrange(
                    "(kt p) (m q) -> p kt m q", p=128, q=128))
            qproj(1)
            attn1_block(1)
            qproj(0)
            attn1_block(0, interleave=(lambda: conv_chain(3, 0),
                                       lambda: conv_chain(3, 1)))
            gather(1, 1, 0)
            blk2_3 = attn2_block(3)
            conv_chain(2, 0)
            conv_chain(2, 1)
            blk2_2 = attn2_block(2, interleave=(lambda: outproj(blk2_3, 3, 0),
                                                lambda: outproj(blk2_3, 3, 1)))
            conv_chain(1, 0)
            conv_chain(1, 1)
            blk2_1 = attn2_block(1, interleave=(lambda: outproj(blk2_2, 2, 0),
                                                lambda: outproj(blk2_2, 2, 1)))
            conv_chain(0, 0)
            conv_chain(0, 1)
            blk2_0 = attn2_block(0, interleave=(lambda: outproj(blk2_1, 1, 0),
                                                lambda: outproj(blk2_1, 1, 1)))
            outproj(blk2_0, 0)

    nc.compile()
    _CACHE[key] = nc
    return nc


def _pow2_scale(arr, target=224.0):
    m = float(np.max(np.abs(arr)))
    if m <= 0:
        return 0
    return int(math.floor(math.log2(target / m)))


def prep_inputs(x, Wqkv_w, Wqkv_b, conv_w, conv_b, out_w, out_b):
    """Build the 8 per-core input maps from the full problem inputs."""
    x = np.asarray(x, np.float32)
    Wqkv_w = np.asarray(Wqkv_w, np.float32)
    Wqkv_b = np.asarray(Wqkv_b, np.float32)
    conv_w = np.asarray(conv_w, np.float32)
    conv_b = np.asarray(conv_b, np.float32)
    out_w = np.asarray(out_w, np.float32)

    scale = 1.0 / np.sqrt(DH).astype(np.float32)
    tri = (np.arange(128)[None, :] >= np.arange(128)[:, None]).astype(np.float32)
    tri2 = np.concatenate([tri, tri], axis=1).astype(BFNP)

    ex = [_pow2_scale(x[b]) for b in range(B)]

    in_maps = []
    for g in range(N_CORES):
        b, hg = g // 4, g % 4
        h0 = HPC * hg
        # q/k row blocks, m-tiles: [q pair0, q pair1, k pair0, k pair1]
        rows = []
        biases = []
        for blk, sc in ((0, scale), (1, 1.0)):
            for pr in range(2):
                r0 = blk * D + (h0 + 2 * pr) * DH
                rows.append(Wqkv_w[r0:r0 + 128, :] * sc)
                biases.append(Wqkv_b[r0:r0 + 128] * sc)
        wqk = np.concatenate(rows, axis=0)  # [512 ch, D]
        eq = _pow2_scale(wqk[0:256])
        ek = _pow2_scale(wqk[256:512])
        wqk_s = wqk * np.concatenate([np.full(256, 2.0 ** eq, np.float32),
                                      np.full(256, 2.0 ** ek, np.float32)])[:, None]
        # [512, D] -> [D, 512] -> [KTP, 2, 128, 512] -> [128, KTP, 2, 512]
        wqk8 = np.ascontiguousarray(
            wqk_s.T.reshape(KTP, 2, 128, 512).transpose(2, 0, 1, 3)
        ).astype(E4NP).reshape(128, KTP * 2 * 512)
        qkb = np.stack(biases, axis=0).astype(np.float32)  # [4, 128]
        c0 = CL * hg
        wv = np.ascontiguousarray(
            Wqkv_w[2 * D + c0:2 * D + c0 + CL, :].T).astype(BFNP)
        vbb = np.ascontiguousarray(
            np.broadcast_to(Wqkv_b[2 * D + c0:2 * D + c0 + CL], (128, CL)))
        # conv weights: [o, i, tap] -> fp8 [128p, tap, ktp, sub, o]
        cwl = conv_w[c0:c0 + CL, :, :] * scale
        ecw = _pow2_scale(cwl)
        cw8 = np.ascontiguousarray(
            (cwl * 2.0 ** ecw).transpose(2, 1, 0)          # [tap, i, o]
            .reshape(3, KTP, 2, 128, CL).transpose(3, 0, 1, 2, 4)
        ).astype(E4NP).reshape(128, 3 * KTP * 2 * CL)
        cb = (conv_b[c0:c0 + CL] * scale).reshape(2, 128).astype(np.float32)
        owm = np.ascontiguousarray(
            out_w[:, c0:c0 + CL].T).astype(BFNP)  # [CL, D]
        dsc_row = np.zeros(NDSC, np.float32)
        dsc_row[DSC_Q] = 2.0 ** (-(ex[b] + eq))
        dsc_row[DSC_K] = 2.0 ** (-(ex[b] + ek))
        dsc_row[DSC_CV] = 2.0 ** (-(ecw + int(math.log2(CTXGS))))
        dsc_row[DSC_LNP8] = math.log(P8S)
        dsc_row[DSC_ZERO] = 0.0
        dsc_row[DSC_V8] = V8S
        dsc = np.ascontiguousarray(np.broadcast_to(dsc_row, (128, NDSC)))
        in_maps.append({
            "xT": np.ascontiguousarray(x[b].T).astype(BFNP),
            "xT8": np.ascontiguousarray(x[b].T * 2.0 ** ex[b]).astype(E4NP),
            "wqk8": wqk8, "wv": wv,
            "qkb": np.ascontiguousarray(qkb),
            "vbb": vbb, "cw8": cw8,
            "cb": np.ascontiguousarray(cb),
            "ow": owm, "tri2": tri2,
            "dsc": dsc,
        })
    return in_maps


def postprocess(results, out_b):
    out_b = np.asarray(out_b, np.float32)
    out = np.empty((B, S, D), np.float32)
    for b in range(B):
        acc = np.zeros((D, S), np.float64)
        for g in GROUPS[b]:
            acc += np.asarray(results[g]["outT"], np.float64)
        out[b] = acc.T.astype(np.float32) + out_b[None, :]
    return out


def kernel(x, Wqkv_w, Wqkv_b, conv_w, conv_b, out_w, out_b):
    nc = build_kernel()
    in_maps = prep_inputs(x, Wqkv_w, Wqkv_b, conv_w, conv_b, out_w, out_b)
    res = run_bass_kernel_spmd(nc, in_maps, core_ids=list(range(N_CORES)))
    return postprocess(res.results, out_b)


# revision 42
# speedup vs baseline: 1.1221x; 1.0233x over previous
"""Trainium2 Bass kernel for nn_MHC (dense transformer block: QKV -> causal
attention -> conv1d(k=3) -> causal attention (same K/V) -> out proj).

Sharding over 8 NeuronCores: data-parallel on batch (2) x tensor-parallel on
heads (16 heads -> 4 per core). Cores 0-3 own batch 0, cores 4-7 batch 1.
Per-token-block AllGather (groups of 4) exchanges attention-1 context (fp8
payload) so each core can run the channel-mixing conv for its own output
channels; gathers issue immediately after each block's normalize and overlap
the next block's attention compute.

fp8 (e4m3) DoubleRow matmuls carry the q/k projections, the conv1d, and the
attention-1 context accumulation (2-4x bf16 PE throughput); scores, v, ctx2
and the out projection stay bf16 for accuracy. Softmax denominators ride as
64 broadcast "ones" rows in each ctx matmul's stationary operand, so
normalization is a PSUM-direct copy/reciprocal/multiply on the vector engine
and the scalar engine runs exp only.
"""

import math

import numpy as np
import ml_dtypes

import concourse.bacc as bacc
import concourse.mybir as mybir
import concourse.tile as tile
from concourse.bass import ts
from concourse.bass_utils import run_bass_kernel_spmd

# Problem shapes (hardcoded per contract)
B, S, D = 2, 2048, 1024
H, DH = 16, 64
N_CORES = 8
HPC = 4          # heads per core
CL = HPC * DH    # 256 local channels
KT = D // 128    # 8 k-tiles over the model dim
KTP = KT // 2    # 4 fp8 double-row k-tile pairs
NJ = S // 512    # 4 t-blocks of 512
NS = S // 128    # 16 s-tiles of 128
GROUPS = [[0, 1, 2, 3], [4, 5, 6, 7]]

F32 = mybir.dt.float32
BF16 = mybir.dt.bfloat16
F8 = mybir.dt.float8e4
EXP = mybir.ActivationFunctionType.Exp
MULT = mybir.AluOpType.mult
ADD = mybir.AluOpType.add
DR = mybir.MatmulPerfMode.DoubleRow

E4NP = ml_dtypes.float8_e4m3
BFNP = ml_dtypes.bfloat16

P8S = 2.0        # fp8 scale on exp(score) in attention 1 (headroom for the
                 # below-diagonal scores that tri masks AFTER exp: fp8
                 # overflow there would turn the masked zeros into NaN)
V8S = 32.0       # fp8 scale on v for the attention-1 ctx matmul
ONE8 = 0.5       # ones-column value in v8
CTXGS = 64.0     # scale of the gathered fp8 ctx: P8S*V8S / (P8S*ONE8)
# ctx1 psum = (8p)(32v) = 256*sum(pv); den rows = (8p)(0.5) = 4*sum(p);
# evacuate-multiply by 1/denrows -> 64 * ctx = CTXGS * ctx.

# dsc columns (per-core dynamic constants, broadcast to 128 partitions)
DSC_Q = 0        # 2^-(ex+eq)*2^eq8: q-proj psum -> fp8 q
DSC_K = 1        # 2^-(ex+ek)*2^ek8: k-proj psum -> fp8 k
DSC_CV = 2       # 2^-(ecw+6)*2^eq28: conv psum -> fp8 q2
DSC_LNP8 = 3     # ln(P8S): exp bias for attention 1
DSC_ZERO = 4     # 0.0: exp bias for attention 2
DSC_V8 = 5       # V8S
DSC_SEXP1 = 6    # 2^-(eq8+ek8): descale fp8 score1 psum inside exp
DSC_SEXP2 = 7    # 2^-(eq28+ek8): descale fp8 score2 psum inside exp
NDSC = 8
EQ28 = 9         # fp8 scale exponent for q2 (|q2|max ~0.29 -> ~147)

_CACHE = {}


def build_kernel(collective=True):
    key = ("nc", collective)
    if key in _CACHE:
        return _CACHE[key]
    nc = bacc.Bacc("TRN2", target_bir_lowering=False, debug=False,
                   num_devices=N_CORES if collective else 1)

    # ---- I/O ----
    xT_d = nc.dram_tensor("xT", [D, S], BF16, kind="ExternalInput")
    xT8_d = nc.dram_tensor("xT8", [D, S], F8, kind="ExternalInput")
    wqk8_d = nc.dram_tensor("wqk8", [128, KTP * 2 * 512], F8, kind="ExternalInput")
    wv_d = nc.dram_tensor("wv", [D, CL], BF16, kind="ExternalInput")
    qkb_d = nc.dram_tensor("qkb", [4, 128], F32, kind="ExternalInput")
    vbb_d = nc.dram_tensor("vbb", [128, CL], F32, kind="ExternalInput")
    cw8_d = nc.dram_tensor("cw8", [128, 3 * KTP * 2 * CL], F8, kind="ExternalInput")
    cb_d = nc.dram_tensor("cb", [2, 128], F32, kind="ExternalInput")
    ow_d = nc.dram_tensor("ow", [CL, D], BF16, kind="ExternalInput")
    tri2_d = nc.dram_tensor("tri2", [128, 256], BF16, kind="ExternalInput")
    dsc_d = nc.dram_tensor("dsc", [128, NDSC], F32, kind="ExternalInput")
    outT_d = nc.dram_tensor("outT", [D, S], BF16, kind="ExternalOutput")

    xT_v = xT_d.ap().rearrange("(kt p) t -> p kt t", p=128)
    xT8_v = xT8_d.ap().rearrange("(ktp sub p) t -> p ktp sub t", p=128, sub=2)
    outT_v = outT_d.ap().rearrange("(m p) t -> p m t", p=128)

    with tile.TileContext(nc) as tc:
        with (
            tc.tile_pool(name="w", bufs=1) as wp,
            tc.tile_pool(name="big", bufs=1) as bigp,
            tc.tile_pool(name="xs", bufs=2) as xsp,
            tc.tile_pool(name="p", bufs=3) as pp,
            tc.tile_pool(name="nrm", bufs=2) as nrmp,
            tc.tile_pool(name="blk", bufs=2) as blkp,
            tc.tile_pool(name="ob", bufs=3) as obp,
            tc.tile_pool(name="ps", bufs=1, space="PSUM") as psp,
            tc.tile_pool(name="dram", bufs=1, space="DRAM") as dramp,
        ):
            # ---- load weights / constants ----
            # wqk8 + the first x8 block lead the DMA queue so the k
            # projection starts ASAP
            wqk8 = wp.tile([128, KTP, 2, 512], F8)
            wqk8_v = wqk8_d.ap().rearrange("p (a s m) -> p a s m", a=KTP, s=2)
            xt8s = [wp.tile([128, KTP, 2, 512], F8, name=f"xt8_{j}")
                    for j in range(NJ)]
            # first two k-tile pairs in separate small DMAs so the first
            # projection matmuls start earlier (deps are range-aware)
            nc.sync.dma_start(wqk8[:, 0:2], wqk8_v[:, 0:2])
            nc.sync.dma_start(xt8s[0][:, 0:2], xT8_v[:, 0:2, :, ts(0, 512)])
            nc.sync.dma_start(wqk8[:, 2:KTP], wqk8_v[:, 2:KTP])
            nc.sync.dma_start(xt8s[0][:, 2:KTP], xT8_v[:, 2:KTP, :, ts(0, 512)])
            dsc = wp.tile([128, NDSC], F32)
            nc.sync.dma_start(dsc[:], dsc_d.ap())
            qkb = wp.tile([128, 4], F32)
            nc.sync.dma_start(qkb[:], qkb_d.ap().rearrange("m p -> p m"))
            wv = wp.tile([128, KT, CL], BF16)
            nc.sync.dma_start(wv[:], wv_d.ap().rearrange("(kt p) c -> p kt c", p=128))
            vbb = wp.tile([128, CL], F32)
            nc.sync.dma_start(vbb[:], vbb_d.ap())
            tri2 = wp.tile([128, 2, 128], BF16)
            nc.sync.dma_start(tri2[:], tri2_d.ap().rearrange("p (h t) -> p h t", h=2))
            tri8 = wp.tile([128, 128], F8)
            nc.vector.tensor_copy(out=tri8[:], in_=tri2[:, 0, :])
            # conv / out-proj weights are DMA'd during attention 1
            cw8 = wp.tile([128, 3, KTP, 2, CL], F8)
            cb = wp.tile([128, 2], F32)
            ow = wp.tile([128, 2, 8, 128], BF16)

            # ---- persistent activations ----
            qpair = bigp.tile([128, 2, S], BF16, name="qpair")
            kpair = bigp.tile([128, 2, S], BF16, name="kpair")
            q2pair = bigp.tile([128, 2, S], BF16, name="q2pair")
            # v for ctx2 (bf16): cols 64:128 are ones -> den rows in psum
            v_sb = bigp.tile([128, NS, HPC, 128], BF16, name="v_sb")
            nc.vector.memset(v_sb[:, :, :, 64:128], 1.0)
            # v for ctx1 (fp8 double-row s-tile pairs): cols 64:128 are ONE8
            v8 = bigp.tile([128, NS // 2, 2, HPC, 128], F8, name="v8")
            nc.vector.memset(v8[:, :, :, :, 64:128], ONE8)
            ctxg = bigp.tile([128, KT, S + 2], F8, name="ctxg")
            nc.vector.memset(ctxg[:, :, 0:1], 0.0)
            nc.vector.memset(ctxg[:, :, S + 1:S + 2], 0.0)

            # gather payloads: blocks (0,1), (2), (3) — ascending processing
            # lets the first gather issue after only ~30% of attention-1
            CCW = [1024, 512, 512]
            cc_in = [dramp.tile([CL, CCW[g]], F8, tag=f"ci{g}", name=f"ci{g}")
                     for g in range(3)]
            cc_out = [dramp.tile([D, CCW[g]], F8, tag=f"co{g}", name=f"co{g}")
                      for g in range(3)]

            # ================= Phase A: K/V projections =================
            for j in range(NJ):
                if j > 0:
                    nc.sync.dma_start(xt8s[j][:], xT8_v[:, :, :, ts(j, 512)])
                xt = xsp.tile([128, KT, 512], BF16, tag="xt", bufs=2)
                nc.sync.dma_start(xt[:], xT_v[:, :, ts(j, 512)])
                # k (m=2,3) via fp8 double-row: m=2 -> dims 0:32 (sub 0),
                # m=3 -> dims 32:64 (sub 1), all 4 heads along partitions
                for m in range(2, 4):
                    ps = psp.tile([128, 512], F32, tag="mm", bufs=2)
                    for kp in range(KTP):
                        nc.tensor.matmul(ps[:], wqk8[:, kp, :, ts(m, 128)],
                                         xt8s[j][:, kp, :, :], perf_mode=DR,
                                         start=(kp == 0), stop=(kp == KTP - 1))
                    nc.vector.tensor_scalar(kpair[:, m % 2, ts(j, 512)], ps[:],
                                            dsc[:, DSC_K:DSC_K + 1],
                                            qkb[:, m:m + 1], MULT, ADD)
                # v token-major (bf16): [t, c] for the 4 s-tiles of this block
                for u in range(4):
                    ps = psp.tile([128, CL], F32, tag="mm", bufs=2)
                    for kt in range(KT):
                        nc.tensor.matmul(ps[:], xt[:, kt, ts(u, 128)],
                                         wv[:, kt, :],
                                         start=(kt == 0), stop=(kt == KT - 1))
                    st_i = 4 * j + u
                    nc.vector.tensor_tensor(
                        v_sb[:, st_i, :, 0:64],
                        ps.rearrange("p (h e) -> p h e", e=64),
                        vbb.rearrange("p (h e) -> p h e", e=64), ADD)
                # fp8 copy of v (scaled by V8S) for the ctx1 double-row
                nc.vector.tensor_scalar(
                    v8[:, 2 * j:2 * j + 2, :, :, 0:64].rearrange(
                        "p a b h d -> p (a b) h d"),
                    v_sb[:, 4 * j:4 * j + 4, :, 0:64],
                    dsc[:, DSC_V8:DSC_V8 + 1], None, MULT)

            # ============ pipelined attention 1 / gather / conv / attn 2 ====
            def qproj(j):
                for m in range(2):
                    ps = psp.tile([128, 512], F32, tag="mm", bufs=2)
                    for kp in range(KTP):
                        nc.tensor.matmul(ps[:], wqk8[:, kp, :, ts(m, 128)],
                                         xt8s[j][:, kp, :, :], perf_mode=DR,
                                         start=(kp == 0), stop=(kp == KTP - 1))
                    nc.vector.tensor_scalar(qpair[:, m, ts(j, 512)], ps[:],
                                            dsc[:, DSC_Q:DSC_Q + 1],
                                            qkb[:, m:m + 1], MULT, ADD)

            def attn1_head(j, h, blk, n_pairs):
                kp, row = h // 2, slice(64 * (h % 2), 64 * (h % 2) + 64)
                cps = psp.tile([128, 512], F32, tag="ctx", bufs=2, name="ctx1")
                pend = None

                def expctx1(stp, pr, c0, c1v, diag):
                    p8t = pp.tile([128, 2, 512], F8, tag="p1")
                    nc.scalar.activation(p8t[:, :, c0:512],
                                         stp[:, :, c0:512], EXP,
                                         bias=dsc[:, DSC_LNP8:DSC_LNP8 + 1])
                    if diag:
                        # zero the below-diagonal strip of subtile 1,
                        # tri-mask both subtiles' diagonal strips
                        nc.gpsimd.memset(p8t[:, 1, c0:c1v], 0.0)
                        nc.vector.tensor_tensor(
                            p8t[:, 0, c0:c0 + 128], p8t[:, 0, c0:c0 + 128],
                            tri8[:], MULT)
                        nc.vector.tensor_tensor(
                            p8t[:, 1, c1v:c1v + 128], p8t[:, 1, c1v:c1v + 128],
                            tri8[:], MULT)
                    nc.tensor.matmul(cps[:, c0:512], v8[:, pr, :, h, :],
                                     p8t[:, :, c0:512], perf_mode=DR,
                                     start=(pr == 0), stop=(pr == n_pairs - 1))

                for pr in range(n_pairs):
                    i0, i1 = 2 * pr, 2 * pr + 1
                    rr = i0 - 4 * j
                    c0 = 128 * rr if rr > 0 else 0
                    c1v = c0 + 128 if rr >= 0 else 0
                    stp = psp.tile([128, 2, 512], F32, tag="st", bufs=2)
                    nc.tensor.matmul(stp[:, 0, c0:512],
                                     kpair[row, kp, ts(i0, 128)],
                                     qpair[row, kp, j * 512 + c0:(j + 1) * 512])
                    nc.tensor.matmul(stp[:, 1, c1v:512],
                                     kpair[row, kp, ts(i1, 128)],
                                     qpair[row, kp, j * 512 + c1v:(j + 1) * 512])
                    if pend is not None:
                        expctx1(*pend)
                    pend = (stp, pr, c0, c1v, rr >= 0)
                expctx1(*pend)
                # normalize: reciprocal of den rows (64:128), multiply the
                # ctx rows -> fp8 block (scale CTXGS)
                dsb = nrmp.tile([64, 512], F32, tag="dsb")
                nc.vector.tensor_copy(out=dsb[:], in_=cps[64:128, :])
                rc = nrmp.tile([64, 512], F32, tag="rc")
                nc.vector.reciprocal_approx_fast(rc[:], dsb[:])
                nc.vector.tensor_tensor(blk[row, kp, :], cps[0:64, :],
                                        rc[:], MULT)

            def attn1_block(j, interleave=()):
                """interleave: per-head callables (or None) run after head
                iterations — fill the PE's exp gaps with independent work."""
                blk = blkp.tile([128, 2, 512], F8, tag="c1")
                inter = list(interleave)
                for h in range(HPC):
                    attn1_head(j, h, blk, 2 * j + 2)
                    if inter:
                        fn = inter.pop(0)
                        if fn is not None:
                            fn()
                g, half = (0, j) if j <= 1 else (j - 1, 0)
                nc.sync.dma_start(
                    cc_in[g].opt()[:, ts(half, 512)].rearrange(
                        "(k p) t -> p k t", p=128),
                    blk[:])

            def gather(g, js):
                if collective:
                    nc.gpsimd.collective_compute(
                        "AllGather", mybir.AluOpType.bypass,
                        replica_groups=GROUPS,
                        ins=[cc_in[g].opt()], outs=[cc_out[g].opt()])
                else:
                    for g4 in range(4):
                        nc.sync.dma_start(
                            cc_out[g].opt()[CL * g4:CL * (g4 + 1), :],
                            cc_in[g].opt()[:])
                for half, j in enumerate(js):
                    nc.sync.dma_start(
                        ctxg[:, :, 1 + j * 512:1 + (j + 1) * 512],
                        cc_out[g].opt()[:, ts(half, 512)].rearrange(
                            "(kt p) t -> p kt t", p=128))

            def conv_chain(j, ot):
                ps = psp.tile([128, 512], F32, tag="mm", bufs=2)
                first = True
                for tap in range(3):
                    for kp in range(KTP):
                        nc.tensor.matmul(
                            ps[:], cw8[:, tap, kp, :, ts(ot, 128)],
                            ctxg[:, 2 * kp:2 * kp + 2,
                                 j * 512 + tap:j * 512 + tap + 512],
                            perf_mode=DR, start=first,
                            stop=(tap == 2 and kp == KTP - 1))
                        first = False
                nc.vector.tensor_scalar(q2pair[:, ot, ts(j, 512)], ps[:],
                                        dsc[:, DSC_CV:DSC_CV + 1],
                                        cb[:, ot:ot + 1], MULT, ADD)

            def attn2_block(j, interleave=()):
                blk2 = blkp.tile([128, 2, 512], BF16, tag="c2")
                i_last = 4 * j + 3
                inter = list(interleave)
                for kp in range(2):
                    cps2 = [psp.tile([128, 512], F32, tag="ctx", bufs=2,
                                     name="ctx2") for _ in range(2)]
                    pend = None

                    def expctx2(st_v, i, c0, cps2=cps2, kp=kp):
                        p = pp.tile([128, 2, 512], BF16, tag="p2")
                        nc.scalar.activation(p[:, :, c0:512],
                                             st_v[:, :, c0:512], EXP,
                                             bias=dsc[:, DSC_ZERO:DSC_ZERO + 1])
                        if i - 4 * j >= 0:
                            nc.vector.tensor_tensor(p[:, :, c0:c0 + 128],
                                                    p[:, :, c0:c0 + 128],
                                                    tri2[:], MULT)
                        for hh in range(2):
                            nc.tensor.matmul(cps2[hh][:, c0:512],
                                             v_sb[:, i, 2 * kp + hh, :],
                                             p[:, hh, c0:512],
                                             start=(i == 0), stop=(i == i_last))

                    for i in range(4 * j + 4):
                        r = i - 4 * j
                        c0 = 128 * r if r > 0 else 0
                        st = psp.tile([128, 2, 512], F32, tag="st", bufs=2)
                        for hh in range(2):
                            rowh = slice(64 * hh, 64 * hh + 64)
                            nc.tensor.matmul(st[:, hh, c0:512],
                                             kpair[rowh, kp, ts(i, 128)],
                                             q2pair[rowh, kp,
                                                    j * 512 + c0:(j + 1) * 512])
                        if pend is not None:
                            expctx2(*pend)
                        pend = (st, i, c0)
                    expctx2(*pend)
                    for hh in range(2):
                        dsb = nrmp.tile([64, 512], F32, tag="dsb")
                        nc.vector.tensor_copy(out=dsb[:], in_=cps2[hh][64:128, :])
                        rc = nrmp.tile([64, 512], F32, tag="rc")
                        nc.vector.reciprocal_approx_fast(rc[:], dsb[:])
                        nc.vector.tensor_tensor(blk2[64 * hh:64 * hh + 64, kp, :],
                                                cps2[hh][0:64, :], rc[:], MULT)
                    if inter:
                        inter.pop(0)()
                return blk2

            def outproj(blk2, j, half=None):
                ms = range(8) if half is None else range(4 * half, 4 * half + 4)
                for m in ms:
                    ps = psp.tile([128, 512], F32, tag="mm", bufs=2)
                    for kt in range(2):
                        nc.tensor.matmul(ps[:], ow[:, kt, m, :],
                                         blk2[:, kt, :],
                                         start=(kt == 0), stop=(kt == 1))
                    ob = obp.tile([128, 512], BF16, tag="ob")
                    nc.vector.tensor_copy(out=ob[:], in_=ps[:])
                    nc.sync.dma_start(outT_v[:, m, ts(j, 512)], ob[:])

            # schedule (ascending): small attn-1 blocks first so the first
            # gather issues early and hides under the big blocks; conv(0)
            # fills attn-1(3)'s tail heads; out-proj trails one block
            qproj(0)
            attn1_block(0)
            qproj(1)
            attn1_block(1)
            gather(0, (0, 1))
            # conv/out-proj weights land during attention-1 compute
            nc.sync.dma_start(
                cw8[:], cw8_d.ap().rearrange(
                    "p (a b s o) -> p a b s o", a=3, b=KTP, s=2))
            nc.sync.dma_start(cb[:], cb_d.ap().rearrange("m p -> p m"))
            nc.sync.dma_start(
                ow[:], ow_d.ap().rearrange(
                    "(kt p) (m q) -> p kt m q", p=128, q=128))
            qproj(2)
            attn1_block(2)
            gather(1, (2,))
            qproj(3)
            attn1_block(3, interleave=(None, None,
                                       lambda: conv_chain(0, 0),
                                       lambda: conv_chain(0, 1)))
            gather(2, (3,))
            blk2_0 = attn2_block(0)
            conv_chain(1, 0)
            conv_chain(1, 1)
            blk2_1 = attn2_block(1, interleave=(lambda: outproj(blk2_0, 0, 0),
                                                lambda: outproj(blk2_0, 0, 1)))
            conv_chain(2, 0)
            conv_chain(2, 1)
            blk2_2 = attn2_block(2, interleave=(lambda: outproj(blk2_1, 1, 0),
                                                lambda: outproj(blk2_1, 1, 1)))
            conv_chain(3, 0)
            conv_chain(3, 1)
            blk2_3 = attn2_block(3, interleave=(lambda: outproj(blk2_2, 2, 0),
                                                lambda: outproj(blk2_2, 2, 1)))
            outproj(blk2_3, 3)

    nc.compile()
    _CACHE[key] = nc
    return nc


def _pow2_scale(arr, target=224.0):
    m = float(np.max(np.abs(arr)))
    if m <= 0:
        return 0
    return int(math.floor(math.log2(target / m)))


def prep_inputs(x, Wqkv_w, Wqkv_b, conv_w, conv_b, out_w, out_b):
    """Build the 8 per-core input maps from the full problem inputs."""
    x = np.asarray(x, np.float32)
    Wqkv_w = np.asarray(Wqkv_w, np.float32)
    Wqkv_b = np.asarray(Wqkv_b, np.float32)
    conv_w = np.asarray(conv_w, np.float32)
    conv_b = np.asarray(conv_b, np.float32)
    out_w = np.asarray(out_w, np.float32)

    scale = 1.0 / np.sqrt(DH).astype(np.float32)
    tri = (np.arange(128)[None, :] >= np.arange(128)[:, None]).astype(np.float32)
    tri2 = np.concatenate([tri, tri], axis=1).astype(BFNP)

    ex = [_pow2_scale(x[b]) for b in range(B)]

    in_maps = []
    for g in range(N_CORES):
        b, hg = g // 4, g % 4
        h0 = HPC * hg
        # q/k row blocks, m-tiles: [q pair0, q pair1, k pair0, k pair1]
        rows = []
        biases = []
        for blk, sc in ((0, scale), (1, 1.0)):
            for pr in range(2):
                r0 = blk * D + (h0 + 2 * pr) * DH
                rows.append(Wqkv_w[r0:r0 + 128, :] * sc)
                biases.append(Wqkv_b[r0:r0 + 128] * sc)
        wqk = np.concatenate(rows, axis=0)  # [512 ch, D]
        eq = _pow2_scale(wqk[0:256])
        ek = _pow2_scale(wqk[256:512])
        wqk_s = wqk * np.concatenate([np.full(256, 2.0 ** eq, np.float32),
                                      np.full(256, 2.0 ** ek, np.float32)])[:, None]
        # [512, D] -> [D, 512] -> [KTP, 2, 128, 512] -> [128, KTP, 2, 512]
        wqk8 = np.ascontiguousarray(
            wqk_s.T.reshape(KTP, 2, 128, 512).transpose(2, 0, 1, 3)
        ).astype(E4NP).reshape(128, KTP * 2 * 512)
        qkb = np.stack(biases, axis=0).astype(np.float32)  # [4, 128]
        c0 = CL * hg
        wv = np.ascontiguousarray(
            Wqkv_w[2 * D + c0:2 * D + c0 + CL, :].T).astype(BFNP)
        vbb = np.ascontiguousarray(
            np.broadcast_to(Wqkv_b[2 * D + c0:2 * D + c0 + CL], (128, CL)))
        # conv weights: [o, i, tap] -> fp8 [128p, tap, ktp, sub, o]
        cwl = conv_w[c0:c0 + CL, :, :] * scale
        ecw = _pow2_scale(cwl)
        cw8 = np.ascontiguousarray(
            (cwl * 2.0 ** ecw).transpose(2, 1, 0)          # [tap, i, o]
            .reshape(3, KTP, 2, 128, CL).transpose(3, 0, 1, 2, 4)
        ).astype(E4NP).reshape(128, 3 * KTP * 2 * CL)
        cb = (conv_b[c0:c0 + CL] * scale).reshape(2, 128).astype(np.float32)
        owm = np.ascontiguousarray(
            out_w[:, c0:c0 + CL].T).astype(BFNP)  # [CL, D]
        dsc_row = np.zeros(NDSC, np.float32)
        dsc_row[DSC_Q] = 2.0 ** (-(ex[b] + eq))
        dsc_row[DSC_K] = 2.0 ** (-(ex[b] + ek))
        dsc_row[DSC_CV] = 2.0 ** (-(ecw + int(math.log2(CTXGS))))
        dsc_row[DSC_LNP8] = math.log(P8S)
        dsc_row[DSC_ZERO] = 0.0
        dsc_row[DSC_V8] = V8S
        dsc = np.ascontiguousarray(np.broadcast_to(dsc_row, (128, NDSC)))
        in_maps.append({
            "xT": np.ascontiguousarray(x[b].T).astype(BFNP),
            "xT8": np.ascontiguousarray(x[b].T * 2.0 ** ex[b]).astype(E4NP),
            "wqk8": wqk8, "wv": wv,
            "qkb": np.ascontiguousarray(qkb),
            "vbb": vbb, "cw8": cw8,
            "cb": np.ascontiguousarray(cb),
            "ow": owm, "tri2": tri2,
            "dsc": dsc,
        })
    return in_maps


def postprocess(results, out_b):
    out_b = np.asarray(out_b, np.float32)
    out = np.empty((B, S, D), np.float32)
    for b in range(B):
        acc = np.zeros((D, S), np.float64)
        for g in GROUPS[b]:
            acc += np.asarray(results[g]["outT"], np.float64)
        out[b] = acc.T.astype(np.float32) + out_b[None, :]
    return out


def kernel(x, Wqkv_w, Wqkv_b, conv_w, conv_b, out_w, out_b):
    nc = build_kernel()
    in_maps = prep_inputs(x, Wqkv_w, Wqkv_b, conv_w, conv_b, out_w, out_b)
    res = run_bass_kernel_spmd(nc, in_maps, core_ids=list(range(N_CORES)))
    return postprocess(res.results, out_b)


# revision 47
# speedup vs baseline: 1.1695x; 1.0423x over previous
"""Trainium2 Bass kernel for nn_MHC (dense transformer block: QKV -> causal
attention -> conv1d(k=3) -> causal attention (same K/V) -> out proj).

Sharding over 8 NeuronCores: data-parallel on batch (2) x tensor-parallel on
heads (16 heads -> 4 per core). Cores 0-3 own batch 0, cores 4-7 batch 1.
Per-token-block AllGather (groups of 4) exchanges attention-1 context (fp8
payload) so each core can run the channel-mixing conv for its own output
channels; gathers issue immediately after each block's normalize and overlap
the next block's attention compute.

fp8 (e4m3) DoubleRow matmuls carry the q/k projections, the conv1d, and the
attention-1 context accumulation (2-4x bf16 PE throughput); scores, v, ctx2
and the out projection stay bf16 for accuracy. Softmax denominators ride as
64 broadcast "ones" rows in each ctx matmul's stationary operand, so
normalization is a PSUM-direct copy/reciprocal/multiply on the vector engine
and the scalar engine runs exp only.
"""

import math

import numpy as np
import ml_dtypes

import concourse.bacc as bacc
import concourse.mybir as mybir
import concourse.tile as tile
from concourse.bass import ts
from concourse.bass_utils import run_bass_kernel_spmd

# Problem shapes (hardcoded per contract)
B, S, D = 2, 2048, 1024
H, DH = 16, 64
N_CORES = 8
HPC = 4          # heads per core
CL = HPC * DH    # 256 local channels
KT = D // 128    # 8 k-tiles over the model dim
KTP = KT // 2    # 4 fp8 double-row k-tile pairs
NJ = S // 512    # 4 t-blocks of 512
NS = S // 128    # 16 s-tiles of 128
GROUPS = [[0, 1, 2, 3], [4, 5, 6, 7]]

F32 = mybir.dt.float32
BF16 = mybir.dt.bfloat16
F8 = mybir.dt.float8e4
EXP = mybir.ActivationFunctionType.Exp
MULT = mybir.AluOpType.mult
ADD = mybir.AluOpType.add
DR = mybir.MatmulPerfMode.DoubleRow

E4NP = ml_dtypes.float8_e4m3
BFNP = ml_dtypes.bfloat16

P8S = 2.0        # fp8 scale on exp(score) in attention 1 (headroom for the
                 # below-diagonal scores that tri masks AFTER exp: fp8
                 # overflow there would turn the masked zeros into NaN)
V8S = 32.0       # fp8 scale on v for the attention-1 ctx matmul
ONE8 = 0.5       # ones-column value in v8
CTXGS = 64.0     # scale of the gathered fp8 ctx: P8S*V8S / (P8S*ONE8)
# ctx1 psum = (8p)(32v) = 256*sum(pv); den rows = (8p)(0.5) = 4*sum(p);
# evacuate-multiply by 1/denrows -> 64 * ctx = CTXGS * ctx.

# dsc columns (per-core dynamic constants, broadcast to 128 partitions)
DSC_Q = 0        # 2^-(ex+eq)*2^eq8: q-proj psum -> fp8 q
DSC_K = 1        # 2^-(ex+ek)*2^ek8: k-proj psum -> fp8 k
DSC_CV = 2       # 2^-(ecw+6)*2^eq28: conv psum -> fp8 q2
DSC_LNP8 = 3     # ln(P8S): exp bias for attention 1
DSC_ZERO = 4     # 0.0: exp bias for attention 2
DSC_V8 = 5       # V8S
DSC_SEXP1 = 6    # 2^-(eq8+ek8): descale fp8 score1 psum inside exp
DSC_SEXP2 = 7    # 2^-(eq28+ek8): descale fp8 score2 psum inside exp
NDSC = 8
EQ28 = 9         # fp8 scale exponent for q2 (|q2|max ~0.29 -> ~147)

_CACHE = {}


def build_kernel(collective=True):
    key = ("nc", collective)
    if key in _CACHE:
        return _CACHE[key]
    nc = bacc.Bacc("TRN2", target_bir_lowering=False, debug=False,
                   num_devices=N_CORES if collective else 1)

    # ---- I/O ----
    xT_d = nc.dram_tensor("xT", [D, S], BF16, kind="ExternalInput")
    xT8_d = nc.dram_tensor("xT8", [D, S], F8, kind="ExternalInput")
    wqk8_d = nc.dram_tensor("wqk8", [128, KTP * 2 * 512], F8, kind="ExternalInput")
    wv_d = nc.dram_tensor("wv", [D, CL], BF16, kind="ExternalInput")
    qkb_d = nc.dram_tensor("qkb", [4, 128], F32, kind="ExternalInput")
    vbb_d = nc.dram_tensor("vbb", [128, CL], F32, kind="ExternalInput")
    cw8_d = nc.dram_tensor("cw8", [128, 3 * KTP * 2 * CL], F8, kind="ExternalInput")
    cb_d = nc.dram_tensor("cb", [2, 128], F32, kind="ExternalInput")
    ow_d = nc.dram_tensor("ow", [CL, D], BF16, kind="ExternalInput")
    tri2_d = nc.dram_tensor("tri2", [128, 256], BF16, kind="ExternalInput")
    dsc_d = nc.dram_tensor("dsc", [128, NDSC], F32, kind="ExternalInput")
    outT_d = nc.dram_tensor("outT", [D, S], BF16, kind="ExternalOutput")

    xT_v = xT_d.ap().rearrange("(kt p) t -> p kt t", p=128)
    xT8_v = xT8_d.ap().rearrange("(ktp sub p) t -> p ktp sub t", p=128, sub=2)
    outT_v = outT_d.ap().rearrange("(m p) t -> p m t", p=128)

    with tile.TileContext(nc) as tc:
        with (
            tc.tile_pool(name="w", bufs=1) as wp,
            tc.tile_pool(name="big", bufs=1) as bigp,
            tc.tile_pool(name="xs", bufs=2) as xsp,
            tc.tile_pool(name="p", bufs=3) as pp,
            tc.tile_pool(name="nrm", bufs=2) as nrmp,
            tc.tile_pool(name="blk", bufs=2) as blkp,
            tc.tile_pool(name="ob", bufs=3) as obp,
            tc.tile_pool(name="ps", bufs=1, space="PSUM") as psp,
            tc.tile_pool(name="dram", bufs=1, space="DRAM") as dramp,
        ):
            # ---- load weights / constants ----
            # wqk8 + the first x8 block lead the DMA queue so the k
            # projection starts ASAP
            wqk8 = wp.tile([128, KTP, 2, 512], F8)
            wqk8_v = wqk8_d.ap().rearrange("p (a s m) -> p a s m", a=KTP, s=2)
            xt8s = [wp.tile([128, KTP, 2, 512], F8, name=f"xt8_{j}")
                    for j in range(NJ)]
            # first two k-tile pairs in separate small DMAs so the first
            # projection matmuls start earlier (deps are range-aware)
            nc.sync.dma_start(wqk8[:, 0:2], wqk8_v[:, 0:2])
            nc.sync.dma_start(xt8s[0][:, 0:2], xT8_v[:, 0:2, :, ts(0, 512)])
            nc.sync.dma_start(wqk8[:, 2:KTP], wqk8_v[:, 2:KTP])
            nc.sync.dma_start(xt8s[0][:, 2:KTP], xT8_v[:, 2:KTP, :, ts(0, 512)])
            dsc = wp.tile([128, NDSC], F32)
            nc.sync.dma_start(dsc[:], dsc_d.ap())
            qkb = wp.tile([128, 4], F32)
            nc.sync.dma_start(qkb[:], qkb_d.ap().rearrange("m p -> p m"))
            wv = wp.tile([128, KT, CL], BF16)
            nc.sync.dma_start(wv[:], wv_d.ap().rearrange("(kt p) c -> p kt c", p=128))
            vbb = wp.tile([128, CL], F32)
            nc.sync.dma_start(vbb[:], vbb_d.ap())
            tri2 = wp.tile([128, 2, 128], BF16)
            nc.sync.dma_start(tri2[:], tri2_d.ap().rearrange("p (h t) -> p h t", h=2))
            tri8 = wp.tile([128, 128], F8)
            nc.vector.tensor_copy(out=tri8[:], in_=tri2[:, 0, :])
            # conv / out-proj weights are DMA'd during attention 1
            cw8 = wp.tile([128, 3, KTP, 2, CL], F8)
            cb = wp.tile([128, 2], F32)
            ow = wp.tile([128, 2, 8, 128], BF16)

            # ---- persistent activations ----
            qpair = bigp.tile([128, 2, S], BF16, name="qpair")
            kpair = bigp.tile([128, 2, S], BF16, name="kpair")
            q2pair = bigp.tile([128, 2, S], BF16, name="q2pair")
            # v for ctx2 (bf16): cols 64:128 are ones -> den rows in psum
            v_sb = bigp.tile([128, NS, HPC, 128], BF16, name="v_sb")
            nc.vector.memset(v_sb[:, :, :, 64:128], 1.0)
            # v for ctx1 (fp8 double-row s-tile pairs): cols 64:128 are ONE8
            v8 = bigp.tile([128, NS // 2, 2, HPC, 128], F8, name="v8")
            nc.vector.memset(v8[:, :, :, :, 64:128], ONE8)
            ctxg = bigp.tile([128, KT, S + 2], F8, name="ctxg")
            nc.vector.memset(ctxg[:, :, 0:1], 0.0)
            nc.vector.memset(ctxg[:, :, S + 1:S + 2], 0.0)

            # gather payloads: blocks (0,1), (2), (3) — ascending processing
            # lets the first gather issue after only ~30% of attention-1
            CCW = [1024, 512, 512]
            cc_in = [dramp.tile([CL, CCW[g]], F8, tag=f"ci{g}", name=f"ci{g}")
                     for g in range(3)]
            cc_out = [dramp.tile([D, CCW[g]], F8, tag=f"co{g}", name=f"co{g}")
                      for g in range(3)]

            # ================= Phase A: K/V projections =================
            for j in range(NJ):
                if j > 0:
                    nc.sync.dma_start(xt8s[j][:], xT8_v[:, :, :, ts(j, 512)])
                xt = xsp.tile([128, KT, 512], BF16, tag="xt", bufs=2)
                nc.sync.dma_start(xt[:], xT_v[:, :, ts(j, 512)])
                # k (m=2,3) via fp8 double-row: m=2 -> dims 0:32 (sub 0),
                # m=3 -> dims 32:64 (sub 1), all 4 heads along partitions
                for m in range(2, 4):
                    ps = psp.tile([128, 512], F32, tag="mm", bufs=2)
                    for kp in range(KTP):
                        nc.tensor.matmul(ps[:], wqk8[:, kp, :, ts(m, 128)],
                                         xt8s[j][:, kp, :, :], perf_mode=DR,
                                         start=(kp == 0), stop=(kp == KTP - 1))
                    nc.vector.tensor_scalar(kpair[:, m % 2, ts(j, 512)], ps[:],
                                            dsc[:, DSC_K:DSC_K + 1],
                                            qkb[:, m:m + 1], MULT, ADD)
                # v token-major (bf16): [t, c] for the 4 s-tiles of this block
                for u in range(4):
                    ps = psp.tile([128, CL], F32, tag="mm", bufs=2)
                    for kt in range(KT):
                        nc.tensor.matmul(ps[:], xt[:, kt, ts(u, 128)],
                                         wv[:, kt, :],
                                         start=(kt == 0), stop=(kt == KT - 1))
                    st_i = 4 * j + u
                    nc.vector.tensor_tensor(
                        v_sb[:, st_i, :, 0:64],
                        ps.rearrange("p (h e) -> p h e", e=64),
                        vbb.rearrange("p (h e) -> p h e", e=64), ADD)
                # fp8 copy of v (scaled by V8S) for the ctx1 double-row
                nc.vector.tensor_scalar(
                    v8[:, 2 * j:2 * j + 2, :, :, 0:64].rearrange(
                        "p a b h d -> p (a b) h d"),
                    v_sb[:, 4 * j:4 * j + 4, :, 0:64],
                    dsc[:, DSC_V8:DSC_V8 + 1], None, MULT)

            # ============ pipelined attention 1 / gather / conv / attn 2 ====
            def qproj(j):
                for m in range(2):
                    ps = psp.tile([128, 512], F32, tag="mm", bufs=2)
                    for kp in range(KTP):
                        nc.tensor.matmul(ps[:], wqk8[:, kp, :, ts(m, 128)],
                                         xt8s[j][:, kp, :, :], perf_mode=DR,
                                         start=(kp == 0), stop=(kp == KTP - 1))
                    nc.vector.tensor_scalar(qpair[:, m, ts(j, 512)], ps[:],
                                            dsc[:, DSC_Q:DSC_Q + 1],
                                            qkb[:, m:m + 1], MULT, ADD)

            def attn1_head(j, h, blk, n_pairs):
                kp, row = h // 2, slice(64 * (h % 2), 64 * (h % 2) + 64)
                cps = psp.tile([128, 512], F32, tag="ctx", bufs=2, name="ctx1")
                pend = None

                def expctx1(stp, pr, c0, c1v, diag):
                    p8t = pp.tile([128, 2, 512], F8, tag="p1")
                    nc.scalar.activation(p8t[:, :, c0:512],
                                         stp[:, :, c0:512], EXP,
                                         bias=dsc[:, DSC_LNP8:DSC_LNP8 + 1])
                    if diag:
                        # tri-mask both subtiles' diagonal strips (subtile
                        # 1's fully-masked strip was pre-set to -30 in PSUM
                        # so exp wrote zeros there)
                        nc.vector.tensor_tensor(
                            p8t[:, 0, c0:c0 + 128], p8t[:, 0, c0:c0 + 128],
                            tri8[:], MULT)
                        nc.vector.tensor_tensor(
                            p8t[:, 1, c1v:c1v + 128], p8t[:, 1, c1v:c1v + 128],
                            tri8[:], MULT)
                    nc.tensor.matmul(cps[:, c0:512], v8[:, pr, :, h, :],
                                     p8t[:, :, c0:512], perf_mode=DR,
                                     start=(pr == 0), stop=(pr == n_pairs - 1))

                for pr in range(n_pairs):
                    i0, i1 = 2 * pr, 2 * pr + 1
                    rr = i0 - 4 * j
                    c0 = 128 * rr if rr > 0 else 0
                    c1v = c0 + 128 if rr >= 0 else 0
                    stp = psp.tile([128, 2, 512], F32, tag="st", bufs=2)
                    if rr >= 0:
                        # subtile 1's fully-masked strip: -30 so exp -> 0
                        # (stale PSUM there could overflow fp8 exp to NaN;
                        # on the vector engine to keep gpsimd free for
                        # collectives)
                        nc.vector.memset(stp[:, 1, c0:c1v], -30.0)
                    nc.tensor.matmul(stp[:, 0, c0:512],
                                     kpair[row, kp, ts(i0, 128)],
                                     qpair[row, kp, j * 512 + c0:(j + 1) * 512])
                    nc.tensor.matmul(stp[:, 1, c1v:512],
                                     kpair[row, kp, ts(i1, 128)],
                                     qpair[row, kp, j * 512 + c1v:(j + 1) * 512])
                    if pend is not None:
                        expctx1(*pend)
                    pend = (stp, pr, c0, c1v, rr >= 0)
                expctx1(*pend)
                # normalize: reciprocal of den rows (64:128), multiply the
                # ctx rows -> fp8 block (scale CTXGS)
                dsb = nrmp.tile([64, 512], F32, tag="dsb")
                nc.vector.tensor_copy(out=dsb[:], in_=cps[64:128, :])
                rc = nrmp.tile([64, 512], F32, tag="rc")
                nc.vector.reciprocal_approx_fast(rc[:], dsb[:])
                nc.vector.tensor_tensor(blk[row, kp, :], cps[0:64, :],
                                        rc[:], MULT)

            def attn1_block(j, interleave=()):
                """interleave: per-head callables (or None) run after head
                iterations — fill the PE's exp gaps with independent work."""
                blk = blkp.tile([128, 2, 512], F8, tag="c1")
                inter = list(interleave)
                for h in range(HPC):
                    attn1_head(j, h, blk, 2 * j + 2)
                    if inter:
                        fn = inter.pop(0)
                        if fn is not None:
                            fn()
                g, half = (0, j) if j <= 1 else (j - 1, 0)
                nc.sync.dma_start(
                    cc_in[g].opt()[:, ts(half, 512)].rearrange(
                        "(k p) t -> p k t", p=128),
                    blk[:])

            def gather(g, js):
                if collective:
                    nc.gpsimd.collective_compute(
                        "AllGather", mybir.AluOpType.bypass,
                        replica_groups=GROUPS,
                        ins=[cc_in[g].opt()], outs=[cc_out[g].opt()])
                else:
                    for g4 in range(4):
                        nc.sync.dma_start(
                            cc_out[g].opt()[CL * g4:CL * (g4 + 1), :],
                            cc_in[g].opt()[:])
                for half, j in enumerate(js):
                    nc.sync.dma_start(
                        ctxg[:, :, 1 + j * 512:1 + (j + 1) * 512],
                        cc_out[g].opt()[:, ts(half, 512)].rearrange(
                            "(kt p) t -> p kt t", p=128))

            def conv_chain(j, ot):
                ps = psp.tile([128, 512], F32, tag="mm", bufs=2)
                first = True
                for tap in range(3):
                    for kp in range(KTP):
                        nc.tensor.matmul(
                            ps[:], cw8[:, tap, kp, :, ts(ot, 128)],
                            ctxg[:, 2 * kp:2 * kp + 2,
                                 j * 512 + tap:j * 512 + tap + 512],
                            perf_mode=DR, start=first,
                            stop=(tap == 2 and kp == KTP - 1))
                        first = False
                nc.vector.tensor_scalar(q2pair[:, ot, ts(j, 512)], ps[:],
                                        dsc[:, DSC_CV:DSC_CV + 1],
                                        cb[:, ot:ot + 1], MULT, ADD)

            def attn2_block(j, interleave=()):
                blk2 = blkp.tile([128, 2, 512], BF16, tag="c2")
                i_last = 4 * j + 3
                inter = list(interleave)
                for kp in range(2):
                    cps2 = [psp.tile([128, 512], F32, tag="ctx", bufs=2,
                                     name="ctx2") for _ in range(2)]
                    pend = None

                    def expctx2(st_v, i, c0, cps2=cps2, kp=kp):
                        p = pp.tile([128, 2, 512], BF16, tag="p2")
                        nc.scalar.activation(p[:, :, c0:512],
                                             st_v[:, :, c0:512], EXP,
                                             bias=dsc[:, DSC_ZERO:DSC_ZERO + 1])
                        if i - 4 * j >= 0:
                            nc.vector.tensor_tensor(p[:, :, c0:c0 + 128],
                                                    p[:, :, c0:c0 + 128],
                                                    tri2[:], MULT)
                        for hh in range(2):
                            nc.tensor.matmul(cps2[hh][:, c0:512],
                                             v_sb[:, i, 2 * kp + hh, :],
                                             p[:, hh, c0:512],
                                             start=(i == 0), stop=(i == i_last))

                    for i in range(4 * j + 4):
                        r = i - 4 * j
                        c0 = 128 * r if r > 0 else 0
                        st = psp.tile([128, 2, 512], F32, tag="st", bufs=2)
                        for hh in range(2):
                            rowh = slice(64 * hh, 64 * hh + 64)
                            nc.tensor.matmul(st[:, hh, c0:512],
                                             kpair[rowh, kp, ts(i, 128)],
                                             q2pair[rowh, kp,
                                                    j * 512 + c0:(j + 1) * 512])
                        if pend is not None:
                            expctx2(*pend)
                        pend = (st, i, c0)
                    expctx2(*pend)
                    for hh in range(2):
                        dsb = nrmp.tile([64, 512], F32, tag="dsb")
                        nc.vector.tensor_copy(out=dsb[:], in_=cps2[hh][64:128, :])
                        rc = nrmp.tile([64, 512], F32, tag="rc")
                        nc.vector.reciprocal_approx_fast(rc[:], dsb[:])
                        nc.vector.tensor_tensor(blk2[64 * hh:64 * hh + 64, kp, :],
                                                cps2[hh][0:64, :], rc[:], MULT)
                    if inter:
                        inter.pop(0)()
                return blk2

            def outproj(blk2, j, half=None):
                ms = range(8) if half is None else range(4 * half, 4 * half + 4)
                for m in ms:
                    ps = psp.tile([128, 512], F32, tag="mm", bufs=2)
                    for kt in range(2):
                        nc.tensor.matmul(ps[:], ow[:, kt, m, :],
                                         blk2[:, kt, :],
                                         start=(kt == 0), stop=(kt == 1))
                    ob = obp.tile([128, 512], BF16, tag="ob")
                    nc.vector.tensor_copy(out=ob[:], in_=ps[:])
                    nc.sync.dma_start(outT_v[:, m, ts(j, 512)], ob[:])

            # schedule (ascending): small attn-1 blocks first so the first
            # gather issues early and hides under the big blocks; conv(0)
            # fills attn-1(3)'s tail heads; out-proj trails one block
            qproj(0)
            attn1_block(0)
            qproj(1)
            attn1_block(1)
            gather(0, (0, 1))
            # conv/out-proj weights land during attention-1 compute
            nc.sync.dma_start(
                cw8[:], cw8_d.ap().rearrange(
                    "p (a b s o) -> p a b s o", a=3, b=KTP, s=2))
            nc.sync.dma_start(cb[:], cb_d.ap().rearrange("m p -> p m"))
            nc.sync.dma_start(
                ow[:], ow_d.ap().rearrange(
                    "(kt p) (m q) -> p kt m q", p=128, q=128))
            qproj(2)
            attn1_block(2)
            gather(1, (2,))
            qproj(3)
            attn1_block(3, interleave=(None, None,
                                       lambda: conv_chain(0, 0),
                                       lambda: conv_chain(0, 1)))
            gather(2, (3,))
            blk2_0 = attn2_block(0)
            conv_chain(1, 0)
            conv_chain(1, 1)
            blk2_1 = attn2_block(1, interleave=(lambda: outproj(blk2_0, 0, 0),
                                                lambda: outproj(blk2_0, 0, 1)))
            conv_chain(2, 0)
            conv_chain(2, 1)
            blk2_2 = attn2_block(2, interleave=(lambda: outproj(blk2_1, 1, 0),
                                                lambda: outproj(blk2_1, 1, 1)))
            conv_chain(3, 0)
            conv_chain(3, 1)
            blk2_3 = attn2_block(3, interleave=(lambda: outproj(blk2_2, 2, 0),
                                                lambda: outproj(blk2_2, 2, 1)))
            outproj(blk2_3, 3)

    nc.compile()
    _CACHE[key] = nc
    return nc


def _pow2_scale(arr, target=224.0):
    m = float(np.max(np.abs(arr)))
    if m <= 0:
        return 0
    return int(math.floor(math.log2(target / m)))


def prep_inputs(x, Wqkv_w, Wqkv_b, conv_w, conv_b, out_w, out_b):
    """Build the 8 per-core input maps from the full problem inputs."""
    x = np.asarray(x, np.float32)
    Wqkv_w = np.asarray(Wqkv_w, np.float32)
    Wqkv_b = np.asarray(Wqkv_b, np.float32)
    conv_w = np.asarray(conv_w, np.float32)
    conv_b = np.asarray(conv_b, np.float32)
    out_w = np.asarray(out_w, np.float32)

    scale = 1.0 / np.sqrt(DH).astype(np.float32)
    tri = (np.arange(128)[None, :] >= np.arange(128)[:, None]).astype(np.float32)
    tri2 = np.concatenate([tri, tri], axis=1).astype(BFNP)

    ex = [_pow2_scale(x[b]) for b in range(B)]

    in_maps = []
    for g in range(N_CORES):
        b, hg = g // 4, g % 4
        h0 = HPC * hg
        # q/k row blocks, m-tiles: [q pair0, q pair1, k pair0, k pair1]
        rows = []
        biases = []
        for blk, sc in ((0, scale), (1, 1.0)):
            for pr in range(2):
                r0 = blk * D + (h0 + 2 * pr) * DH
                rows.append(Wqkv_w[r0:r0 + 128, :] * sc)
                biases.append(Wqkv_b[r0:r0 + 128] * sc)
        wqk = np.concatenate(rows, axis=0)  # [512 ch, D]
        eq = _pow2_scale(wqk[0:256])
        ek = _pow2_scale(wqk[256:512])
        wqk_s = wqk * np.concatenate([np.full(256, 2.0 ** eq, np.float32),
                                      np.full(256, 2.0 ** ek, np.float32)])[:, None]
        # [512, D] -> [D, 512] -> [KTP, 2, 128, 512] -> [128, KTP, 2, 512]
        wqk8 = np.ascontiguousarray(
            wqk_s.T.reshape(KTP, 2, 128, 512).transpose(2, 0, 1, 3)
        ).astype(E4NP).reshape(128, KTP * 2 * 512)
        qkb = np.stack(biases, axis=0).astype(np.float32)  # [4, 128]
        c0 = CL * hg
        wv = np.ascontiguousarray(
            Wqkv_w[2 * D + c0:2 * D + c0 + CL, :].T).astype(BFNP)
        vbb = np.ascontiguousarray(
            np.broadcast_to(Wqkv_b[2 * D + c0:2 * D + c0 + CL], (128, CL)))
        # conv weights: [o, i, tap] -> fp8 [128p, tap, ktp, sub, o]
        cwl = conv_w[c0:c0 + CL, :, :] * scale
        ecw = _pow2_scale(cwl)
        cw8 = np.ascontiguousarray(
            (cwl * 2.0 ** ecw).transpose(2, 1, 0)          # [tap, i, o]
            .reshape(3, KTP, 2, 128, CL).transpose(3, 0, 1, 2, 4)
        ).astype(E4NP).reshape(128, 3 * KTP * 2 * CL)
        cb = (conv_b[c0:c0 + CL] * scale).reshape(2, 128).astype(np.float32)
        owm = np.ascontiguousarray(
            out_w[:, c0:c0 + CL].T).astype(BFNP)  # [CL, D]
        dsc_row = np.zeros(NDSC, np.float32)
        dsc_row[DSC_Q] = 2.0 ** (-(ex[b] + eq))
        dsc_row[DSC_K] = 2.0 ** (-(ex[b] + ek))
        dsc_row[DSC_CV] = 2.0 ** (-(ecw + int(math.log2(CTXGS))))
        dsc_row[DSC_LNP8] = math.log(P8S)
        dsc_row[DSC_ZERO] = 0.0
        dsc_row[DSC_V8] = V8S
        dsc = np.ascontiguousarray(np.broadcast_to(dsc_row, (128, NDSC)))
        in_maps.append({
            "xT": np.ascontiguousarray(x[b].T).astype(BFNP),
            "xT8": np.ascontiguousarray(x[b].T * 2.0 ** ex[b]).astype(E4NP),
            "wqk8": wqk8, "wv": wv,
            "qkb": np.ascontiguousarray(qkb),
            "vbb": vbb, "cw8": cw8,
            "cb": np.ascontiguousarray(cb),
            "ow": owm, "tri2": tri2,
            "dsc": dsc,
        })
    return in_maps


def postprocess(results, out_b):
    out_b = np.asarray(out_b, np.float32)
    out = np.empty((B, S, D), np.float32)
    for b in range(B):
        acc = np.zeros((D, S), np.float64)
        for g in GROUPS[b]:
            acc += np.asarray(results[g]["outT"], np.float64)
        out[b] = acc.T.astype(np.float32) + out_b[None, :]
    return out


def kernel(x, Wqkv_w, Wqkv_b, conv_w, conv_b, out_w, out_b):
    nc = build_kernel()
    in_maps = prep_inputs(x, Wqkv_w, Wqkv_b, conv_w, conv_b, out_w, out_b)
    res = run_bass_kernel_spmd(nc, in_maps, core_ids=list(range(N_CORES)))
    return postprocess(res.results, out_b)
